# revision 31
# baseline (speedup 1.0000x reference)
"""Trainium2 Bass kernel for nn_DecoderBlock (attention + top-2 MoE), 8 cores.

Sharding:
  - Attention: tensor-parallel over heads (2 Q heads + their KV head per
    core); per-head context is exchanged with a small bf16 AllToAll so each
    core applies the full Wo to its own 256 token rows locally (no big
    ReduceScatter of [T, D] partials).
  - Router: replicated math on each core's token rows (fp32 matmuls).
  - MoE: pair-wise sharding. Cores {2g, 2g+1} share a 512-token block;
    each core runs 4 of the 8 experts densely over the block (scaled by
    the top-2 combine weight, zero if not routed). h+comb are AllGathered
    only within the pair, and a pair ReduceScatter sums the two cores'
    expert contributions back to each core's 256 token rows. This keeps
    expert flops identical to 1-expert-per-core but shrinks the two MoE
    collectives from all-8 broadcast volume to pair-local volume.
Precision:
  - Attention matmuls run as float32r (full-speed PE mode, ~1.5e-4 rel err),
    router matmul in plain fp32, expert FFN in bf16 (weights host-cast).
  - All three collectives (attn ReduceScatter, h AllGather, expert-output
    ReduceScatter) carry bf16 payloads: collective wire time dominates the
    on-device cost, and halving the bytes keeps rel err ~1.3e-3 (<< 2e-2).
"""
import os
import sys

import numpy as np

for _p in ("/opt/trn_rl_repo", "/root/.axon_site/_ro/trn_rl_repo"):
    if os.path.isdir(_p) and _p not in sys.path:
        sys.path.append(_p)

import ml_dtypes  # noqa: E402

import concourse.bacc as bacc  # noqa: E402
import concourse.bass as bass  # noqa: E402
import concourse.tile as tile  # noqa: E402
from concourse import mybir  # noqa: E402
from concourse.bass_utils import run_bass_kernel_spmd  # noqa: E402

F32 = mybir.dt.float32
F32R = mybir.dt.float32r
BF16 = mybir.dt.bfloat16
AX = mybir.AxisListType
ALU = mybir.AluOpType
ACTF = mybir.ActivationFunctionType

T = 2048          # tokens
D = 2048          # model dim
P = 128           # partitions
NT = T // P       # 16 token tiles
ND = D // P       # 16 dim chunks
HD = 128          # head dim
NQ = 16           # query heads
NE = 8            # experts
EH = 4096         # expert hidden
NEH = EH // P     # 32
NCORES = 8
RT = T // NCORES  # 256 rows per core
NRT = RT // P     # 2
EPC = 4           # experts per core (pair-wise MoE sharding)
PT = 2 * RT       # 512 tokens per core pair
EPS = 1e-6
ROPE_BASE = 5e6
NEG = -1e9
SM_SCALE = 1.0 / float(np.sqrt(HD))
HPC = NQ // NCORES   # 2 q heads per core


def _pbcast(ap, p=P):
    """AP that broadcasts a [1, ...] source across p partitions (DMA only)."""
    return bass.AP(tensor=ap.tensor, offset=ap.offset,
                   ap=[[0, p]] + [list(x) for x in ap.ap[1:]])


def _build():
    nc = bacc.Bacc()

    dp = nc.declare_dram_parameter
    x_full = dp("x_full", [T, D], F32, isOutput=False)
    x_rows = dp("x_rows", [RT, D], F32, isOutput=False)
    wqkv = dp("wqkv", [D, 512], F32R, isOutput=False)      # [Wq 2 heads | Wk | Wv]
    wo_full = dp("wo_full", [D, D], BF16, isOutput=False)   # full Wo (bf16)
    wgate = dp("wgate", [D, NE], F32, isOutput=False)
    anw = dp("anw", [1, D], F32, isOutput=False)
    fnw = dp("fnw", [1, D], F32, isOutput=False)
    qnw = dp("qnw", [1, HD], F32, isOutput=False)
    knw = dp("knw", [1, HD], F32, isOutput=False)
    cos_t = dp("cos_t", [T, HD], F32, isOutput=False)
    sin_t = dp("sin_t", [T, HD], F32, isOutput=False)
    tri01 = dp("tri01", [P, P], F32, isOutput=False)
    ident = dp("ident", [P, P], F32, isOutput=False)
    identb = dp("identb", [P, P], BF16, isOutput=False)
    esel4 = dp("esel4", [EPC, NE], F32, isOutput=False)
    onesr = dp("onesr", [P, 1], F32R, isOutput=False)
    wi_e = dp("wi_e", [EPC, NEH, P, ND, P], BF16, isOutput=False)
    wg_e = dp("wg_e", [EPC, NEH, P, ND, P], BF16, isOutput=False)
    woe = dp("woe", [EPC, ND, P, NEH, P], BF16, isOutput=False)

    out_r = dp("out_r", [RT, D], F32, isOutput=True)
    debug = bool(int(os.environ.get("DECODER_DEBUG", "0")))
    plimit = int(os.environ.get("DECODER_PHASE_LIMIT", "3"))
    if debug:
        xmid_dbg = dp("xmid_dbg", [RT, D], F32, isOutput=True)
        comb_dbg = dp("comb_dbg", [RT, NE], F32, isOutput=True)
        lgt_dbg = dp("lgt_dbg", [RT, NE], F32, isOutput=True)

    a2a_in = nc.dram_tensor("a2a_in", [T, RT], BF16)
    a2a_out = nc.dram_tensor("a2a_out", [T, RT], BF16)
    hcomb = nc.dram_tensor("hcomb", [RT, D + NE], BF16)
    hcomb_all = nc.dram_tensor("hcomb_all", [PT, D + NE], BF16)
    ybuf = nc.dram_tensor("ybuf", [PT, D], BF16)
    rs2 = nc.dram_tensor("rs2", [RT, D], BF16)
    RG = [list(range(NCORES))]
    RGP = [[2 * g, 2 * g + 1] for g in range(NCORES // 2)]

    repeat = int(os.environ.get("DECODER_REPEAT", "1"))
    hwloop = int(os.environ.get("DECODER_HWLOOP", "0"))
    trace_sim = bool(int(os.environ.get("DECODER_TRACE_SIM", "0")))
    from contextlib import nullcontext

    with tile.TileContext(nc, trace_sim=trace_sim) as tc:
      with (tc.For_i(0, hwloop, 1) if hwloop else nullcontext()):
       for _rep in range(repeat):
        with (
            tc.tile_pool(name=f"consts{_rep}", bufs=1) as cp,
            tc.tile_pool(name=f"xmid{_rep}", bufs=1) as xp,
        ):
            c_ident = cp.tile([P, P], F32, tag="ident")
            nc.sync.dma_start(out=c_ident, in_=ident[:])
            c_tri = cp.tile([P, P], F32, tag="tri")
            nc.sync.dma_start(out=c_tri, in_=tri01[:])
            c_anw = cp.tile([P, D], F32, tag="anw")
            nc.gpsimd.dma_start(out=c_anw, in_=_pbcast(anw[:]))
            c_fnw = cp.tile([P, D], F32, tag="fnw")
            nc.gpsimd.dma_start(out=c_fnw, in_=_pbcast(fnw[:]))
            c_qnw = cp.tile([P, HD], F32, tag="qnw")
            nc.gpsimd.dma_start(out=c_qnw, in_=_pbcast(qnw[:]))
            c_knw = cp.tile([P, HD], F32, tag="knw")
            nc.gpsimd.dma_start(out=c_knw, in_=_pbcast(knw[:]))
            c_esel4 = cp.tile([P, EPC, NE], F32, tag="esel4")
            for _i in range(EPC):
                nc.gpsimd.dma_start(out=c_esel4[:, _i, :],
                                    in_=_pbcast(esel4[_i:_i + 1, :]))
            c_wgate = cp.tile([P, ND, NE], F32, tag="wgate")
            nc.sync.dma_start(out=c_wgate,
                              in_=wgate.rearrange("(c p) e -> p c e", p=P))
            c_ones = cp.tile([P, 1], F32R, tag="ones")
            nc.sync.dma_start(out=c_ones, in_=onesr[:])
            c_eps = cp.tile([P, 1], F32, tag="eps")
            nc.vector.memset(c_eps, EPS)
            c_ones1 = cp.tile([1, P], F32, tag="ones1")
            nc.vector.memset(c_ones1, 1.0)

            x_mid = xp.tile([P, NRT, D], F32, tag="xmid")

            # qT/kT/vv/ctxT survive phases A..C
            if plimit == 4:
                pass
            else:
             with tc.tile_pool(name="qkv_keep", bufs=1) as pk:
                qT = pk.tile([P, HPC, T], F32R, tag="qT")    # [hd, head, tok]
                kT = pk.tile([P, T], F32R, tag="kT")         # [hd, tok]
                vv = pk.tile([P, NT, HD], F32R, tag="vv")    # [tok, kt, hd]
                ctxT = pk.tile([P, HPC, T], F32R, tag="ctxT")

                # ---------------- Phase A: rmsnorm + QKV projection ----------
                with (
                    tc.tile_pool(name="pa2", bufs=2) as pa2,
                    tc.tile_pool(name="pa1", bufs=1) as pa1,
                    tc.tile_pool(name="pas", bufs=2) as pas,
                    tc.tile_pool(name="pa_ps", bufs=2, space="PSUM") as paps,
                    tc.tile_pool(name="pa_ps2", bufs=2, space="PSUM") as paps2,
                ):
                    c_cos = pa1.tile([P, NT, HD], F32, tag="cos")
                    nc.sync.dma_start(out=c_cos,
                                      in_=cos_t.rearrange("(t p) d -> p t d", p=P))
                    c_sin = pa1.tile([P, NT, HD], F32, tag="sin")
                    nc.sync.dma_start(out=c_sin,
                                      in_=sin_t.rearrange("(t p) d -> p t d", p=P))
                    w_qkv = pa1.tile([P, ND, 512], F32R, tag="wqkv")
                    nc.sync.dma_start(out=w_qkv,
                                      in_=wqkv.rearrange("(c p) n -> p c n", p=P))
                    scr = pa1.tile([P, D], F32, tag="scr")

                    for tt in range(NT):
                        xt = pa2.tile([P, D], F32, tag="xt")
                        nc.sync.dma_start(out=xt, in_=x_full[tt * P:(tt + 1) * P, :])
                        ms = pas.tile([P, 1], F32, tag="ms")
                        nc.scalar.activation(out=scr, in_=xt, func=ACTF.Square,
                                             accum_out=ms)
                        nc.scalar.activation(out=ms, in_=ms, func=ACTF.Sqrt,
                                             bias=c_eps, scale=1.0 / D)
                        nc.vector.reciprocal(out=ms, in_=ms)
                        at = pa2.tile([P, D], F32, tag="at")
                        nc.vector.scalar_tensor_tensor(
                            out=at, in0=xt, scalar=ms, in1=c_anw,
                            op0=ALU.mult, op1=ALU.mult)
                        aT = pa1.tile([P, ND, P], F32R, tag="aT")
                        for dc in range(ND):
                            tp = paps.tile([P, P], F32, tag="tp")
                            nc.tensor.transpose(out=tp,
                                                in_=at[:, dc * P:(dc + 1) * P],
                                                identity=c_ident)
                            nc.vector.tensor_copy(out=aT[:, dc, :], in_=tp)
                        qkvp = paps2.tile([P, 512], F32, tag="qkvp")
                        for dc in range(ND):
                            nc.tensor.matmul(out=qkvp[:],
                                             lhsT=aT[:, dc, :],
                                             rhs=w_qkv[:, dc, :],
                                             start=(dc == 0), stop=(dc == ND - 1))
                        # q heads + k: per-head rmsnorm + rope, then transpose
                        for ih in range(HPC + 1):
                            seg = qkvp[:, ih * HD:(ih + 1) * HD]
                            wnorm = c_qnw if ih < HPC else c_knw
                            scr2 = pas.tile([P, HD], F32, tag="scr2")
                            ms2 = pas.tile([P, 1], F32, tag="ms2")
                            nc.scalar.activation(out=scr2, in_=seg,
                                                 func=ACTF.Square, accum_out=ms2)
                            nc.scalar.activation(out=ms2, in_=ms2, func=ACTF.Sqrt,
                                                 bias=c_eps, scale=1.0 / HD)
                            nc.vector.reciprocal(out=ms2, in_=ms2)
                            nrm = pas.tile([P, HD], F32, tag="nrm")
                            nc.vector.scalar_tensor_tensor(
                                out=nrm, in0=seg, scalar=ms2, in1=wnorm,
                                op0=ALU.mult, op1=ALU.mult)
                            rop = pas.tile([P, HD], F32, tag="rop")
                            nc.vector.tensor_scalar_mul(
                                rop[:, :HD // 2], nrm[:, HD // 2:], -1.0)
                            nc.vector.tensor_copy(
                                out=rop[:, HD // 2:], in_=nrm[:, :HD // 2])
                            nc.vector.tensor_mul(nrm, nrm, c_cos[:, tt, :])
                            nc.vector.tensor_mul(rop, rop, c_sin[:, tt, :])
                            nc.vector.tensor_add(nrm, nrm, rop)
                            tp2 = paps.tile([P, P], F32, tag="tp")
                            nc.tensor.transpose(out=tp2, in_=nrm, identity=c_ident)
                            dst = (qT[:, ih, tt * P:(tt + 1) * P] if ih < HPC
                                   else kT[:, tt * P:(tt + 1) * P])
                            nc.vector.tensor_copy(out=dst, in_=tp2)
                        nc.vector.tensor_copy(out=vv[:, tt, :], in_=qkvp[:, 384:512])

                # ---------------- Phase B: attention ----------------------
                with (
                    tc.tile_pool(name="pb", bufs=3) as pb,
                    tc.tile_pool(name="pb2", bufs=2) as pb2,
                    tc.tile_pool(name="pb_ps", bufs=2, space="PSUM") as pbps,
                    tc.tile_pool(name="pb_ps2", bufs=2, space="PSUM") as pbps2,
                    tc.tile_pool(name="pb_ps3", bufs=1, space="PSUM") as pbps3,
                ):
                    for h in range(HPC):
                        for qc in range(4):
                            cs = qc * 512
                            ctxp = pbps2.tile([P, 512], F32, tag="ctx")
                            denp = pbps3.tile([1, 512], F32, tag="den")
                            nkt = 4 * (qc + 1)
                            for kt in range(nkt):
                                lo = max(0, kt * P - cs)
                                width = 512 - lo
                                scp = pbps.tile([P, 512], F32, tag="sc")
                                nc.tensor.matmul(
                                    out=scp[:, :width],
                                    lhsT=kT[:, kt * P:(kt + 1) * P],
                                    rhs=qT[:, h, cs + lo:cs + 512],
                                    start=True, stop=True)
                                ex = pb.tile([P, 512], F32R, tag="ex")
                                nc.scalar.activation(out=ex[:, :width],
                                                     in_=scp[:, :width],
                                                     func=ACTF.Exp, scale=SM_SCALE)
                                if kt * P >= cs:
                                    # diagonal block: first 128 cols of suffix
                                    nc.vector.tensor_mul(ex[:, :P], ex[:, :P],
                                                         c_tri)
                                nc.tensor.matmul(
                                    out=ctxp[:, lo:],
                                    lhsT=vv[:, kt, :],
                                    rhs=ex[:, :width],
                                    start=(kt == 0), stop=(kt == nkt - 1))
                                nc.tensor.matmul(
                                    out=denp[:, lo:], lhsT=c_ones,
                                    rhs=ex[:, :width],
                                    start=(kt == 0), stop=(kt == nkt - 1))
                            dsb = pb2.tile([1, 512], F32, tag="dsb")
                            nc.vector.reciprocal(out=dsb, in_=denp)
                            dbc = pbps3.tile([P, 512], F32, tag="dbc")
                            nc.tensor.matmul(out=dbc[:], lhsT=c_ones1, rhs=dsb,
                                             start=True, stop=True)
                            dbc_sb = pb2.tile([P, 512], F32, tag="dbcsb")
                            nc.scalar.copy(out=dbc_sb, in_=dbc)
                            nc.vector.tensor_mul(ctxT[:, h, cs:cs + 512],
                                                 ctxp, dbc_sb)

                # ------- Phase C: export ctx^T (bf16) for the all-to-all ------
                with tc.tile_pool(name="pc", bufs=3) as pc:
                    for j in range(NCORES):
                        for h in range(HPC):
                            cxb = pc.tile([P, RT], BF16, tag="cxb")
                            nc.vector.tensor_copy(
                                out=cxb, in_=ctxT[:, h, j * RT:(j + 1) * RT])
                            nc.sync.dma_start(
                                out=a2a_in[j * RT + h * P:
                                           j * RT + (h + 1) * P, :],
                                in_=cxb)

            if plimit != 4:
                nc.gpsimd.collective_compute(
                    "AllToAll", ALU.bypass, replica_groups=RG,
                    ins=[a2a_in[:]], outs=[a2a_out[:]])

                # ------- Phase C2: x_mid = x_rows + ctx_rows @ Wo -------------
                with (
                    tc.tile_pool(name="pc2", bufs=2) as pc2,
                    tc.tile_pool(name="pc21", bufs=1) as pc21,
                    tc.tile_pool(name="pc2_ps", bufs=2, space="PSUM") as pc2ps,
                    tc.tile_pool(name="pc2_ps2", bufs=2, space="PSUM") as pc2ps2,
                ):
                    wo_sb = pc21.tile([P, ND, D], BF16, tag="wosb")
                    nc.sync.dma_start(
                        out=wo_sb,
                        in_=wo_full.rearrange("(c p) o -> p c o", p=P))
                    ctx_sb = pc21.tile([P, ND, RT], BF16, tag="ctxsb")
                    nc.sync.dma_start(
                        out=ctx_sb,
                        in_=a2a_out.rearrange("(c p) t -> p c t", p=P))
                    xr2 = pc21.tile([P, NRT, D], F32, tag="xr2")
                    nc.sync.dma_start(
                        out=xr2, in_=x_rows.rearrange("(r p) d -> p r d", p=P))
                    for do in range(ND):
                        op_ = pc2ps.tile([P, RT], F32, tag="op")
                        for dc in range(ND):
                            nc.tensor.matmul(
                                out=op_[:],
                                lhsT=wo_sb[:, dc, do * P:(do + 1) * P],
                                rhs=ctx_sb[:, dc, :],
                                start=(dc == 0), stop=(dc == ND - 1))
                        ot_sb = pc2.tile([P, RT], F32, tag="otsb")
                        nc.vector.tensor_copy(out=ot_sb, in_=op_)
                        for r in range(NRT):
                            tp = pc2ps2.tile([P, P], F32, tag="tp2")
                            nc.tensor.transpose(
                                out=tp, in_=ot_sb[:, r * P:(r + 1) * P],
                                identity=c_ident)
                            nc.vector.tensor_add(
                                x_mid[:, r, do * P:(do + 1) * P],
                                xr2[:, r, do * P:(do + 1) * P], tp)

            if plimit >= 2 and plimit != 4:

                # ---------------- Phase D: residual, h, router ----------------
                with (
                    tc.tile_pool(name="pd", bufs=2) as pd,
                    tc.tile_pool(name="pd1", bufs=1) as pd1,
                    tc.tile_pool(name="pd_ps", bufs=2, space="PSUM") as pdps,
                    tc.tile_pool(name="pd_ps2", bufs=1, space="PSUM") as pdps2,
                ):
                    h_sb = pd1.tile([P, NRT, D], F32, tag="hsb")
                    hT_c = pd1.tile([P, ND, RT], F32, tag="hTc")
                    scr3 = pd1.tile([P, D], F32, tag="scr3")
                    for r in range(NRT):
                        ms = pd.tile([P, 1], F32, tag="ms")
                        nc.scalar.activation(out=scr3, in_=x_mid[:, r, :],
                                             func=ACTF.Square, accum_out=ms)
                        nc.scalar.activation(out=ms, in_=ms, func=ACTF.Sqrt,
                                             bias=c_eps, scale=1.0 / D)
                        nc.vector.reciprocal(out=ms, in_=ms)
                        nc.vector.scalar_tensor_tensor(
                            out=h_sb[:, r, :], in0=x_mid[:, r, :], scalar=ms,
                            in1=c_fnw, op0=ALU.mult, op1=ALU.mult)
                        h16 = pd.tile([P, D], BF16, tag="h16")
                        nc.vector.tensor_copy(out=h16, in_=h_sb[:, r, :])
                        nc.sync.dma_start(out=hcomb[r * P:(r + 1) * P, 0:D],
                                          in_=h16)
                        for dc in range(ND):
                            tp = pdps.tile([P, P], F32, tag="tp")
                            nc.tensor.transpose(out=tp,
                                                in_=h_sb[:, r, dc * P:(dc + 1) * P],
                                                identity=c_ident)
                            nc.vector.tensor_copy(out=hT_c[:, dc, r * P:(r + 1) * P],
                                                  in_=tp)
                    # router logits (plain fp32 matmuls, exact)
                    lgp = pdps2.tile([NE, RT], F32, tag="lgp")
                    for dc in range(ND):
                        nc.tensor.matmul(out=lgp[:], lhsT=c_wgate[:, dc, :],
                                         rhs=hT_c[:, dc, :],
                                         start=(dc == 0), stop=(dc == ND - 1))
                    lg_sb = pd1.tile([NE, RT], F32, tag="lgsb")
                    nc.vector.tensor_copy(out=lg_sb, in_=lgp)
                    lg_t = pd1.tile([P, NRT, NE], F32, tag="lgt")
                    for r in range(NRT):
                        tp = pdps.tile([P, NE], F32, tag="tpl")
                        nc.tensor.transpose(out=tp, in_=lg_sb[:, r * P:(r + 1) * P],
                                            identity=c_ident[:NE, :NE])
                        nc.vector.tensor_copy(out=lg_t[:, r, :], in_=tp)
                    for r in range(NRT):
                        row = lg_t[:, r, :]
                        mx = pd.tile([P, 8], F32, tag="mx")
                        nc.vector.max(out=mx, in_=row)
                        nm1 = pd.tile([P, 1], F32, tag="nm1")
                        nc.vector.tensor_scalar_mul(nm1, mx[:, 0:1], -1.0)
                        g = pd.tile([P, NE], F32, tag="g")
                        d8 = pd.tile([P, 1], F32, tag="d8")
                        nc.scalar.activation(out=g, in_=row, func=ACTF.Exp,
                                             bias=nm1, accum_out=d8)
                        nc.vector.reciprocal(out=d8, in_=d8)
                        nc.vector.tensor_scalar_mul(g, g, d8)
                        mg = pd.tile([P, 8], F32, tag="mg")
                        nc.vector.max(out=mg, in_=g)
                        msk = pd.tile([P, NE], F32, tag="msk")
                        nc.vector.tensor_scalar(out=msk, in0=g, scalar1=mg[:, 1:2],
                                                scalar2=None, op0=ALU.is_ge)
                        comb = pd.tile([P, NE], F32, tag="comb")
                        nc.vector.tensor_mul(comb, g, msk)
                        cb16 = pd.tile([P, NE], BF16, tag="cb16")
                        nc.vector.tensor_copy(out=cb16, in_=comb)
                        nc.sync.dma_start(out=hcomb[r * P:(r + 1) * P, D:D + NE],
                                          in_=cb16)
                        if debug:
                            nc.sync.dma_start(out=comb_dbg[r * P:(r + 1) * P, :],
                                              in_=comb)
                            nc.sync.dma_start(out=lgt_dbg[r * P:(r + 1) * P, :],
                                              in_=lg_t[:, r, :])
                            nc.sync.dma_start(out=xmid_dbg[r * P:(r + 1) * P, :],
                                              in_=x_mid[:, r, :])

                nc.gpsimd.collective_compute(
                    "AllGather", ALU.bypass, replica_groups=RGP,
                    ins=[hcomb[:]], outs=[hcomb_all[:]])

            if plimit == 1:
                with tc.tile_pool(name="px1", bufs=2) as px1:
                    for r in range(NRT):
                        nc.sync.dma_start(out=out_r[r * P:(r + 1) * P, :],
                                          in_=x_mid[:, r, :])
            if plimit == 2:
                with tc.tile_pool(name="px2", bufs=2) as px2:
                    for r in range(NRT):
                        t2 = px2.tile([P, D], BF16, tag="t2")
                        nc.sync.dma_start(out=t2,
                                          in_=hcomb_all[r * P:(r + 1) * P, 0:D])
                        t2f = px2.tile([P, D], F32, tag="t2f")
                        nc.vector.tensor_copy(out=t2f, in_=t2)
                        nc.sync.dma_start(out=out_r[r * P:(r + 1) * P, :],
                                          in_=t2f)
            if plimit >= 3:
                # ------- Phase E: 4 experts x 512 pair-local tokens ----------
                with (
                    tc.tile_pool(name="pe1", bufs=1) as pe1,
                    tc.tile_pool(name="pew", bufs=3) as pew,
                    tc.tile_pool(name="pes", bufs=2) as pes,
                    tc.tile_pool(name="pes1", bufs=1) as pes1,
                    tc.tile_pool(name="pe_ps", bufs=2, space="PSUM") as peps,
                    tc.tile_pool(name="pe_ps2", bufs=2, space="PSUM") as peps2,
                    tc.tile_pool(name="pe_ps3", bufs=2, space="PSUM") as peps3,
                ):
                    c_identb = pe1.tile([P, P], BF16, tag="identb")
                    nc.sync.dma_start(out=c_identb, in_=identb[:])
                    NJ = PT // P    # 4 token tiles of 128
                    hT_g = pe1.tile([P, ND, PT], BF16, tag="hTg")
                    act_g = pe1.tile([P, NEH, PT], BF16, tag="actg")
                    combg = pe1.tile([P, NJ, EPC], F32, tag="combg")
                    yacc = pe1.tile([P, NJ, D], F32, tag="yacc")
                    for j in range(NJ):
                        tt0 = j * P
                        hb = pes1.tile([P, D], BF16, tag="hb")
                        nc.sync.dma_start(out=hb, in_=hcomb_all[tt0:tt0 + P, 0:D])
                        for dc in range(ND):
                            tp = peps.tile([P, 512], BF16, tag="peab")
                            nc.tensor.transpose(out=tp[:, :P],
                                                in_=hb[:, dc * P:(dc + 1) * P],
                                                identity=c_identb)
                            nc.vector.tensor_copy(
                                out=hT_g[:, dc, j * P:(j + 1) * P],
                                in_=tp[:, :P])
                        cbl = pes.tile([P, NE], BF16, tag="cbl")
                        nc.sync.dma_start(out=cbl,
                                          in_=hcomb_all[tt0:tt0 + P, D:D + NE])
                        for ei in range(EPC):
                            cbm = pes.tile([P, NE], F32, tag="cbm")
                            nc.vector.tensor_mul(cbm, cbl, c_esel4[:, ei, :])
                            nc.vector.tensor_reduce(
                                out=combg[:, j, ei:ei + 1], in_=cbm,
                                axis=AX.X, op=ALU.add)
                    for ei in range(EPC):
                        for et in range(NEH):
                            wi_s = pew.tile([P, ND, P], BF16, tag="wis")
                            nc.sync.dma_start(out=wi_s, in_=wi_e[ei, et])
                            wg_s = pew.tile([P, ND, P], BF16, tag="wgs")
                            nc.sync.dma_start(out=wg_s, in_=wg_e[ei, et])
                            upp = peps3.tile([P, 512], F32, tag="upp")
                            gtp = peps2.tile([P, 512], F32, tag="peb")
                            for dc in range(ND):
                                nc.tensor.matmul(
                                    out=upp[:], lhsT=wi_s[:, dc, :],
                                    rhs=hT_g[:, dc, :],
                                    start=(dc == 0), stop=(dc == ND - 1))
                            for dc in range(ND):
                                nc.tensor.matmul(
                                    out=gtp[:], lhsT=wg_s[:, dc, :],
                                    rhs=hT_g[:, dc, :],
                                    start=(dc == 0), stop=(dc == ND - 1))
                            sil = pes.tile([P, 512], BF16, tag="sil")
                            nc.scalar.activation(out=sil, in_=gtp, func=ACTF.Silu)
                            nc.vector.tensor_tensor(
                                out=act_g[:, et, :], in0=sil, in1=upp,
                                op=ALU.mult)
                        for dt in range(ND):
                            wo_s = pew.tile([P, NEH, P], BF16, tag="wos")
                            nc.sync.dma_start(out=wo_s, in_=woe[ei, dt])
                            yp = peps.tile([P, 512], F32, tag="pea")
                            for ec in range(NEH):
                                nc.tensor.matmul(
                                    out=yp[:], lhsT=wo_s[:, ec, :],
                                    rhs=act_g[:, ec, :],
                                    start=(ec == 0), stop=(ec == NEH - 1))
                            ysb = pes.tile([P, 512], F32, tag="ysb")
                            nc.vector.tensor_copy(out=ysb, in_=yp)
                            for q in range(NJ):
                                tp = peps2.tile([P, 512], F32, tag="peb")
                                nc.tensor.transpose(
                                    out=tp[:, :P], in_=ysb[:, q * P:(q + 1) * P],
                                    identity=c_ident)
                                dst = yacc[:, q, dt * P:(dt + 1) * P]
                                if ei == 0:
                                    nc.vector.tensor_scalar_mul(
                                        dst, tp[:, :P], combg[:, q, 0:1])
                                else:
                                    nc.vector.scalar_tensor_tensor(
                                        out=dst, in0=tp[:, :P],
                                        scalar=combg[:, q, ei:ei + 1],
                                        in1=dst, op0=ALU.mult, op1=ALU.add)
                    for j in range(NJ):
                        y16 = pes.tile([P, D], BF16, tag="y16")
                        nc.vector.tensor_copy(out=y16, in_=yacc[:, j, :])
                        nc.sync.dma_start(out=ybuf[j * P:(j + 1) * P, :],
                                          in_=y16)

                if plimit != 4:
                    nc.gpsimd.collective_compute(
                        "ReduceScatter", ALU.add, replica_groups=RGP,
                        ins=[ybuf[:]], outs=[rs2[:]])

                # ---------------- Phase F: final residual ---------------------
                with tc.tile_pool(name="pf", bufs=2) as pf:
                    for r in range(NRT):
                        rr = pf.tile([P, D], BF16, tag="rr2")
                        src_t = ybuf if plimit == 4 else rs2
                        nc.sync.dma_start(out=rr, in_=src_t[r * P:(r + 1) * P, :])
                        ot = pf.tile([P, D], F32, tag="ot")
                        if plimit == 4:
                            nc.vector.tensor_copy(out=ot, in_=rr)
                            nc.sync.dma_start(out=out_r[r * P:(r + 1) * P, :],
                                              in_=ot)
                        else:
                            nc.vector.tensor_add(ot, x_mid[:, r, :], rr)
                            nc.sync.dma_start(out=out_r[r * P:(r + 1) * P, :],
                                              in_=ot)


    nc.finalize()
    return nc, debug


_PROG = None


def _get_prog():
    global _PROG
    if _PROG is None:
        _PROG = _build()
    return _PROG


def _rope_tables():
    inv_freq = 1.0 / (ROPE_BASE ** (np.arange(0, HD, 2, dtype=np.float32) / HD))
    t = np.arange(T, dtype=np.float32)
    freqs = np.einsum("i,j->ij", t, inv_freq).astype(np.float32)
    emb = np.concatenate((freqs, freqs), axis=-1)
    return np.cos(emb).astype(np.float32), np.sin(emb).astype(np.float32)


def _wtile_in(w):
    """[D, EH] -> [NEH, P, ND, P] bf16: contiguous per-et lhsT strips."""
    return np.ascontiguousarray(
        w.reshape(ND, P, NEH, P).transpose(2, 1, 0, 3)
    ).astype(ml_dtypes.bfloat16)


def _wtile_out(w):
    """[EH, D] -> [ND, P, NEH, P] bf16: contiguous per-dt lhsT strips."""
    return np.ascontiguousarray(
        w.reshape(NEH, P, ND, P).transpose(2, 1, 0, 3)
    ).astype(ml_dtypes.bfloat16)


_PREP_CACHE = {}


def _make_in_maps(inputs):
    x = np.ascontiguousarray(np.asarray(inputs["x"], np.float32).reshape(T, D))
    mask = np.asarray(inputs["attn_mask"], np.float32).reshape(T, T)
    causal = np.triu(np.full((T, T), NEG, np.float32), k=1)
    if not np.array_equal(mask, causal):
        raise NotImplementedError("kernel compiled for the causal attn_mask")

    Wq = np.asarray(inputs["Wq"], np.float32)
    Wk = np.asarray(inputs["Wk"], np.float32)
    Wv = np.asarray(inputs["Wv"], np.float32)
    Wo = np.asarray(inputs["Wo"], np.float32)
    wi = np.asarray(inputs["wi"], np.float32)
    wg = np.asarray(inputs["wg"], np.float32)
    wo = np.asarray(inputs["wo"], np.float32)
    cos_np, sin_np = _rope_tables()
    tri = np.triu(np.ones((P, P), np.float32))           # [k, q]: 1 if q >= k
    ident_np = np.eye(P, dtype=np.float32)

    key = (np.asarray(inputs["wi"]).ctypes.data,
           np.asarray(inputs["x"]).ctypes.data)
    cached = _PREP_CACHE.get(key)
    if cached is not None:
        return cached
    wi_all = np.stack([_wtile_in(wi[e]) for e in range(NE)])
    wg_all = np.stack([_wtile_in(wg[e]) for e in range(NE)])
    wo_all = np.stack([_wtile_out(wo[e]) for e in range(NE)])
    Wo_b16 = np.ascontiguousarray(Wo).astype(ml_dtypes.bfloat16)
    in_maps = []
    for c in range(NCORES):
        g = c // 2
        wqkv_c = np.ascontiguousarray(np.concatenate(
            [Wq[:, 2 * c * HD:(2 * c + 2) * HD],
             Wk[:, g * HD:(g + 1) * HD],
             Wv[:, g * HD:(g + 1) * HD]], axis=1))
        e0 = EPC * (c % 2)
        esel4_c = np.zeros((EPC, NE), np.float32)
        for i in range(EPC):
            esel4_c[i, e0 + i] = 1.0
        in_maps.append({
            "x_full": x,
            "x_rows": np.ascontiguousarray(x[c * RT:(c + 1) * RT, :]),
            "wqkv": wqkv_c,
            "wo_full": Wo_b16,
            "wgate": np.ascontiguousarray(np.asarray(inputs["w_gate"],
                                                     np.float32)),
            "anw": np.asarray(inputs["attn_norm_w"], np.float32).reshape(1, D),
            "fnw": np.asarray(inputs["ffn_norm_w"], np.float32).reshape(1, D),
            "qnw": np.asarray(inputs["q_norm_w"], np.float32).reshape(1, HD),
            "knw": np.asarray(inputs["k_norm_w"], np.float32).reshape(1, HD),
            "cos_t": cos_np,
            "sin_t": sin_np,
            "tri01": tri,
            "ident": ident_np,
            "identb": ident_np.astype(ml_dtypes.bfloat16),
            "esel4": esel4_c,
            "onesr": np.ones((P, 1), np.float32),
            "wi_e": wi_all[e0:e0 + EPC],
            "wg_e": wg_all[e0:e0 + EPC],
            "woe": wo_all[e0:e0 + EPC],
        })
    return in_maps


_RUNNER = None


def _get_runner():
    """Persistent jitted SPMD executor (compiles once per process)."""
    global _RUNNER
    if _RUNNER is None:
        import jax
        from jax.experimental.shard_map import shard_map
        from jax.sharding import Mesh, PartitionSpec

        from concourse import bass2jax as b2j

        nc, debug = _get_prog()
        b2j.install_neuronx_cc_hook()
        pname = nc.partition_id_tensor.name if nc.partition_id_tensor else None
        in_names, out_names, out_avals, zero_specs = [], [], [], []
        for alloc in nc.m.functions[0].allocations:
            if not isinstance(alloc, mybir.MemoryLocationSet):
                continue
            name = alloc.memorylocations[0].name
            if alloc.kind == "ExternalInput":
                if name != pname:
                    in_names.append(name)
            elif alloc.kind == "ExternalOutput":
                out_names.append(name)
                shape = tuple(alloc.tensor_shape)
                dt_np = mybir.dt.np(alloc.dtype)
                out_avals.append(jax.core.ShapedArray(shape, dt_np))
                zero_specs.append((shape, dt_np))
        n_params = len(in_names)
        all_in = list(in_names) + list(out_names) + ([pname] if pname else [])
        donate = tuple(range(n_params, n_params + len(out_names)))

        def _body(*args):
            operands = list(args)
            if pname is not None:
                operands.append(b2j.partition_id_tensor())
            outs = b2j._bass_exec_p.bind(
                *operands, out_avals=tuple(out_avals), in_names=tuple(all_in),
                out_names=tuple(out_names), lowering_input_output_aliases=(),
                sim_require_finite=True, sim_require_nnan=True, nc=nc)
            return tuple(outs)

        devices = jax.devices()[:NCORES]
        mesh = Mesh(np.asarray(devices), ("core",))
        nio = n_params + len(out_names)
        sharded = jax.jit(
            shard_map(_body, mesh=mesh, in_specs=(PartitionSpec("core"),) * nio,
                      out_specs=(PartitionSpec("core"),) * len(out_names),
                      check_rep=False),
            donate_argnums=donate, keep_unused=True)
        _RUNNER = (sharded, in_names, out_names, zero_specs, debug)
    return _RUNNER


def _run(in_maps):
    sharded, in_names, out_names, zero_specs, debug = _get_runner()
    concat_in = [
        np.concatenate([np.asarray(in_maps[c][nm]) for c in range(NCORES)],
                       axis=0)
        for nm in in_names
    ]
    zeros = [np.zeros((NCORES * s[0],) + tuple(s[1:]), d)
             for (s, d) in zero_specs]
    outs = sharded(*concat_in, *zeros)
    return {nm: np.asarray(outs[i]) for i, nm in enumerate(out_names)}, debug


def kernel(**inputs):
    in_maps = _make_in_maps(inputs)
    res, debug = _run(in_maps)
    out = res["out_r"]  # [NCORES*RT, D] = [T, D], rank-concat = token order
    if debug:
        kernel._dbg = res
    return out.reshape(1, T, D).astype(np.float32)



# revision 50
# speedup vs baseline: 1.0180x; 1.0180x over previous
"""Trainium2 Bass kernel for nn_DecoderBlock (attention + top-2 MoE), 8 cores.

Sharding:
  - Attention: tensor-parallel over heads (2 Q heads + their KV head per
    core); per-head context is exchanged with a small bf16 AllToAll so each
    core applies the full Wo to its own 256 token rows locally (no big
    ReduceScatter of [T, D] partials).
  - Router: replicated math on each core's token rows (fp32 matmuls).
  - MoE: pair-wise sharding. Cores {2g, 2g+1} share a 512-token block;
    each core runs 4 of the 8 experts densely over the block (scaled by
    the top-2 combine weight, zero if not routed). h+comb are AllGathered
    only within the pair, and a pair ReduceScatter sums the two cores'
    expert contributions back to each core's 256 token rows. This keeps
    expert flops identical to 1-expert-per-core but shrinks the two MoE
    collectives from all-8 broadcast volume to pair-local volume.
Precision:
  - Attention matmuls run as float32r (full-speed PE mode, ~1.5e-4 rel err),
    router matmul in plain fp32, expert FFN in bf16 (weights host-cast).
  - All three collectives (attn ReduceScatter, h AllGather, expert-output
    ReduceScatter) carry bf16 payloads: collective wire time dominates the
    on-device cost, and halving the bytes keeps rel err ~1.3e-3 (<< 2e-2).
"""
import os
import sys

import numpy as np

for _p in ("/opt/trn_rl_repo", "/root/.axon_site/_ro/trn_rl_repo"):
    if os.path.isdir(_p) and _p not in sys.path:
        sys.path.append(_p)

import ml_dtypes  # noqa: E402

import concourse.bacc as bacc  # noqa: E402
import concourse.bass as bass  # noqa: E402
import concourse.tile as tile  # noqa: E402
from concourse import mybir  # noqa: E402
from concourse.bass_utils import run_bass_kernel_spmd  # noqa: E402

F32 = mybir.dt.float32
F32R = mybir.dt.float32r
BF16 = mybir.dt.bfloat16
AX = mybir.AxisListType
ALU = mybir.AluOpType
ACTF = mybir.ActivationFunctionType

T = 2048          # tokens
D = 2048          # model dim
P = 128           # partitions
NT = T // P       # 16 token tiles
ND = D // P       # 16 dim chunks
HD = 128          # head dim
NQ = 16           # query heads
NE = 8            # experts
EH = 4096         # expert hidden
NEH = EH // P     # 32
NCORES = 8
RT = T // NCORES  # 256 rows per core
NRT = RT // P     # 2
EPC = 4           # experts per core (pair-wise MoE sharding)
PT = 2 * RT       # 512 tokens per core pair
EPS = 1e-6
ROPE_BASE = 5e6
NEG = -1e9
SM_SCALE = 1.0 / float(np.sqrt(HD))
HPC = NQ // NCORES   # 2 q heads per core


def _pbcast(ap, p=P):
    """AP that broadcasts a [1, ...] source across p partitions (DMA only)."""
    return bass.AP(tensor=ap.tensor, offset=ap.offset,
                   ap=[[0, p]] + [list(x) for x in ap.ap[1:]])


def _build():
    nc = bacc.Bacc()

    dp = nc.declare_dram_parameter
    x_full = dp("x_full", [T, D], F32, isOutput=False)
    x_rows = dp("x_rows", [RT, D], F32, isOutput=False)
    wqkv = dp("wqkv", [D, 512], F32R, isOutput=False)      # [Wq 2 heads | Wk | Wv]
    wo_full = dp("wo_full", [D, D], BF16, isOutput=False)   # full Wo (bf16)
    wgate = dp("wgate", [D, NE], F32, isOutput=False)
    anw = dp("anw", [1, D], F32, isOutput=False)
    fnw = dp("fnw", [1, D], F32, isOutput=False)
    qnw = dp("qnw", [1, HD], F32, isOutput=False)
    knw = dp("knw", [1, HD], F32, isOutput=False)
    cos_t = dp("cos_t", [T, HD], F32, isOutput=False)
    sin_t = dp("sin_t", [T, HD], F32, isOutput=False)
    tri01 = dp("tri01", [P, P], F32, isOutput=False)
    ident = dp("ident", [P, P], F32, isOutput=False)
    identb = dp("identb", [P, P], BF16, isOutput=False)
    esel4 = dp("esel4", [EPC, NE], F32, isOutput=False)
    onesr = dp("onesr", [P, 1], F32R, isOutput=False)
    wi_e = dp("wi_e", [EPC, NEH, P, ND, P], BF16, isOutput=False)
    wg_e = dp("wg_e", [EPC, NEH, P, ND, P], BF16, isOutput=False)
    woe = dp("woe", [EPC, ND, P, NEH, P], BF16, isOutput=False)

    out_r = dp("out_r", [RT, D], F32, isOutput=True)
    debug = bool(int(os.environ.get("DECODER_DEBUG", "0")))
    plimit = int(os.environ.get("DECODER_PHASE_LIMIT", "3"))
    if debug:
        xmid_dbg = dp("xmid_dbg", [RT, D], F32, isOutput=True)
        comb_dbg = dp("comb_dbg", [RT, NE], F32, isOutput=True)
        lgt_dbg = dp("lgt_dbg", [RT, NE], F32, isOutput=True)

    a2a_in = nc.dram_tensor("a2a_in", [T, RT], BF16)
    a2a_out = nc.dram_tensor("a2a_out", [T, RT], BF16)
    hcomb = nc.dram_tensor("hcomb", [RT, D + NE], BF16)
    hcomb_all = nc.dram_tensor("hcomb_all", [PT, D + NE], BF16)
    ybuf = nc.dram_tensor("ybuf", [PT, D], BF16)
    rs2 = nc.dram_tensor("rs2", [RT, D], BF16)
    RG = [list(range(NCORES))]
    RGP = [[2 * g, 2 * g + 1] for g in range(NCORES // 2)]

    repeat = int(os.environ.get("DECODER_REPEAT", "1"))
    hwloop = int(os.environ.get("DECODER_HWLOOP", "0"))
    trace_sim = bool(int(os.environ.get("DECODER_TRACE_SIM", "0")))
    from contextlib import nullcontext

    with tile.TileContext(nc, trace_sim=trace_sim) as tc:
      with (tc.For_i(0, hwloop, 1) if hwloop else nullcontext()):
       for _rep in range(repeat):
        with (
            tc.tile_pool(name=f"consts{_rep}", bufs=1) as cp,
            tc.tile_pool(name=f"xmid{_rep}", bufs=1) as xp,
        ):
            c_ident = cp.tile([P, P], F32, tag="ident")
            nc.sync.dma_start(out=c_ident, in_=ident[:])
            c_tri = cp.tile([P, P], F32, tag="tri")
            nc.sync.dma_start(out=c_tri, in_=tri01[:])
            c_anw = cp.tile([P, D], F32, tag="anw")
            nc.gpsimd.dma_start(out=c_anw, in_=_pbcast(anw[:]))
            c_fnw = cp.tile([P, D], F32, tag="fnw")
            nc.gpsimd.dma_start(out=c_fnw, in_=_pbcast(fnw[:]))
            c_qnw = cp.tile([P, HD], F32, tag="qnw")
            nc.gpsimd.dma_start(out=c_qnw, in_=_pbcast(qnw[:]))
            c_knw = cp.tile([P, HD], F32, tag="knw")
            nc.gpsimd.dma_start(out=c_knw, in_=_pbcast(knw[:]))
            c_esel4 = cp.tile([P, EPC, NE], F32, tag="esel4")
            for _i in range(EPC):
                nc.gpsimd.dma_start(out=c_esel4[:, _i, :],
                                    in_=_pbcast(esel4[_i:_i + 1, :]))
            c_wgate = cp.tile([P, ND, NE], F32, tag="wgate")
            nc.sync.dma_start(out=c_wgate,
                              in_=wgate.rearrange("(c p) e -> p c e", p=P))
            c_ones = cp.tile([P, 1], F32R, tag="ones")
            nc.sync.dma_start(out=c_ones, in_=onesr[:])
            c_eps = cp.tile([P, 1], F32, tag="eps")
            nc.vector.memset(c_eps, EPS)
            c_ones1 = cp.tile([1, P], F32, tag="ones1")
            nc.vector.memset(c_ones1, 1.0)

            x_mid = xp.tile([P, NRT, D], F32, tag="xmid")
            from contextlib import ExitStack
            pwo_ctx = ExitStack()

            # qT/kT/vv/ctxT survive phases A..C
            if plimit == 4:
                pass
            else:
             with tc.tile_pool(name="qkv_keep", bufs=1) as pk:
                qT = pk.tile([P, HPC, T], F32R, tag="qT")    # [hd, head, tok]
                kT = pk.tile([P, T], F32R, tag="kT")         # [hd, tok]
                vv = pk.tile([P, NT, HD], F32R, tag="vv")    # [tok, kt, hd]
                ctxT = pk.tile([P, HPC, T], F32R, tag="ctxT")

                # ---------------- Phase A: rmsnorm + QKV projection ----------
                with (
                    tc.tile_pool(name="pa2", bufs=2) as pa2,
                    tc.tile_pool(name="pa1", bufs=1) as pa1,
                    tc.tile_pool(name="pas", bufs=2) as pas,
                    tc.tile_pool(name="pa_ps", bufs=2, space="PSUM") as paps,
                    tc.tile_pool(name="pa_ps2", bufs=3, space="PSUM") as paps2,
                ):
                    c_cos = pa1.tile([P, NT, HD], F32, tag="cos")
                    nc.sync.dma_start(out=c_cos,
                                      in_=cos_t.rearrange("(t p) d -> p t d", p=P))
                    c_sin = pa1.tile([P, NT, HD], F32, tag="sin")
                    nc.sync.dma_start(out=c_sin,
                                      in_=sin_t.rearrange("(t p) d -> p t d", p=P))
                    w_qkv = pa1.tile([P, ND, 512], F32R, tag="wqkv")
                    nc.sync.dma_start(out=w_qkv,
                                      in_=wqkv.rearrange("(c p) n -> p c n", p=P))
                    scr = pa1.tile([P, D], F32, tag="scr")

                    def _at_chain(tt):
                        # rmsnorm-scaled row tile; issued one tile ahead, and
                        # applied on the ACT engine (attn_norm_w is folded into
                        # the QKV weights host-side) so the wide apply doesn't
                        # clog the in-order DVE queue.
                        xt = pa2.tile([P, D], F32, tag="xt")
                        nc.sync.dma_start(out=xt,
                                          in_=x_full[tt * P:(tt + 1) * P, :])
                        ms = pas.tile([P, 1], F32, tag="ms")
                        nc.scalar.activation(out=scr, in_=xt, func=ACTF.Square,
                                             accum_out=ms)
                        nc.scalar.activation(out=ms, in_=ms, func=ACTF.Sqrt,
                                             bias=c_eps, scale=1.0 / D)
                        nc.vector.reciprocal(out=ms, in_=ms)
                        at = pa2.tile([P, D], F32, tag="at")
                        nc.scalar.activation(out=at, in_=xt, func=ACTF.Copy,
                                             scale=ms)
                        return at

                    at_cur = _at_chain(0)
                    for tt in range(NT):
                        aT = pa1.tile([P, ND, P], F32R, tag="aT")
                        for dq in range(4):
                            tp4 = paps.tile([P, 4, P], F32, tag="tp")
                            for k in range(4):
                                dc = dq * 4 + k
                                nc.tensor.transpose(
                                    out=tp4[:, k, :],
                                    in_=at_cur[:, dc * P:(dc + 1) * P],
                                    identity=c_ident)
                            nc.vector.tensor_copy(
                                out=aT[:, dq * 4:(dq + 1) * 4, :], in_=tp4)
                        qkvp = paps2.tile([P, 512], F32, tag="qkvp")
                        for dc in range(ND):
                            nc.tensor.matmul(out=qkvp[:],
                                             lhsT=aT[:, dc, :],
                                             rhs=w_qkv[:, dc, :],
                                             start=(dc == 0), stop=(dc == ND - 1))
                        if tt + 1 < NT:
                            at_next = _at_chain(tt + 1)
                        # q heads + k: per-head rmsnorm + rope, then transpose
                        for ih in range(HPC + 1):
                            seg = qkvp[:, ih * HD:(ih + 1) * HD]
                            wnorm = c_qnw if ih < HPC else c_knw
                            scr2 = pas.tile([P, HD], F32, tag="scr2")
                            ms2 = pas.tile([P, 1], F32, tag="ms2")
                            nc.scalar.activation(out=scr2, in_=seg,
                                                 func=ACTF.Square, accum_out=ms2)
                            nc.scalar.activation(out=ms2, in_=ms2,
                                                 func=ACTF.Sqrt,
                                                 bias=c_eps, scale=1.0 / HD)
                            nc.vector.reciprocal(out=ms2, in_=ms2)
                            nrm = pas.tile([P, HD], F32, tag="nrm")
                            nc.vector.scalar_tensor_tensor(
                                out=nrm, in0=seg, scalar=ms2, in1=wnorm,
                                op0=ALU.mult, op1=ALU.mult)
                            rop = pas.tile([P, HD], F32, tag="rop")
                            nc.vector.tensor_scalar_mul(
                                rop[:, :HD // 2], nrm[:, HD // 2:], -1.0)
                            nc.vector.tensor_copy(
                                out=rop[:, HD // 2:], in_=nrm[:, :HD // 2])
                            nc.vector.tensor_mul(nrm, nrm, c_cos[:, tt, :])
                            nc.vector.tensor_mul(rop, rop, c_sin[:, tt, :])
                            nc.vector.tensor_add(nrm, nrm, rop)
                            tp2 = paps.tile([P, P], F32, tag="tp")
                            nc.tensor.transpose(out=tp2, in_=nrm, identity=c_ident)
                            dst = (qT[:, ih, tt * P:(tt + 1) * P] if ih < HPC
                                   else kT[:, tt * P:(tt + 1) * P])
                            nc.vector.tensor_copy(out=dst, in_=tp2)
                        nc.vector.tensor_copy(out=vv[:, tt, :], in_=qkvp[:, 384:512])
                        if tt + 1 < NT:
                            at_cur = at_next

                # prefetch phase-C2 operands while attention runs
                # (SBUF for these frees up when the phase-A pools close)
                if plimit != 4:
                    pwo = pwo_ctx.enter_context(
                        tc.tile_pool(name=f"pwo{_rep}", bufs=1))
                    wo_sb = pwo.tile([P, ND, D], BF16, tag="wosb")
                    nc.sync.dma_start(
                        out=wo_sb,
                        in_=wo_full.rearrange("(c p) o -> p c o", p=P))
                    xr2 = pwo.tile([P, NRT, D], F32, tag="xr2")
                    nc.sync.dma_start(
                        out=xr2, in_=x_rows.rearrange("(r p) d -> p r d", p=P))

                # ---------------- Phase B: attention ----------------------
                with (
                    tc.tile_pool(name="pb", bufs=3) as pb,
                    tc.tile_pool(name="pb2", bufs=2) as pb2,
                    tc.tile_pool(name="pb_ps", bufs=2, space="PSUM") as pbps,
                    tc.tile_pool(name="pb_ps2", bufs=2, space="PSUM") as pbps2,
                    tc.tile_pool(name="pb_ps3", bufs=1, space="PSUM") as pbps3,
                ):
                    for h in range(HPC):
                        for qc in range(4):
                            cs = qc * 512
                            ctxp = pbps2.tile([P, 512], F32, tag="ctx")
                            denp = pbps3.tile([1, 512], F32, tag="den")
                            nkt = 4 * (qc + 1)
                            for kt in range(nkt):
                                lo = max(0, kt * P - cs)
                                width = 512 - lo
                                scp = pbps.tile([P, 512], F32, tag="sc")
                                nc.tensor.matmul(
                                    out=scp[:, :width],
                                    lhsT=kT[:, kt * P:(kt + 1) * P],
                                    rhs=qT[:, h, cs + lo:cs + 512],
                                    start=True, stop=True)
                                ex = pb.tile([P, 512], F32R, tag="ex")
                                nc.scalar.activation(out=ex[:, :width],
                                                     in_=scp[:, :width],
                                                     func=ACTF.Exp, scale=SM_SCALE)
                                if kt * P >= cs:
                                    # diagonal block: first 128 cols of suffix
                                    nc.vector.tensor_mul(ex[:, :P], ex[:, :P],
                                                         c_tri)
                                nc.tensor.matmul(
                                    out=ctxp[:, lo:],
                                    lhsT=vv[:, kt, :],
                                    rhs=ex[:, :width],
                                    start=(kt == 0), stop=(kt == nkt - 1))
                                nc.tensor.matmul(
                                    out=denp[:, lo:], lhsT=c_ones,
                                    rhs=ex[:, :width],
                                    start=(kt == 0), stop=(kt == nkt - 1))
                            dsb = pb2.tile([1, 512], F32, tag="dsb")
                            nc.vector.reciprocal(out=dsb, in_=denp)
                            dbc = pbps3.tile([P, 512], F32, tag="dbc")
                            nc.tensor.matmul(out=dbc[:], lhsT=c_ones1, rhs=dsb,
                                             start=True, stop=True)
                            dbc_sb = pb2.tile([P, 512], F32, tag="dbcsb")
                            nc.scalar.copy(out=dbc_sb, in_=dbc)
                            nc.vector.tensor_mul(ctxT[:, h, cs:cs + 512],
                                                 ctxp, dbc_sb)
                            # stream ctx^T out for the all-to-all as soon as
                            # this 512-token chunk of the head is final
                            for j2 in range(2):
                                j = qc * 2 + j2
                                cxb = pb.tile([P, RT], BF16, tag="cxb")
                                nc.vector.tensor_copy(
                                    out=cxb,
                                    in_=ctxT[:, h, j * RT:(j + 1) * RT])
                                nc.sync.dma_start(
                                    out=a2a_in[j * RT + h * P:
                                               j * RT + (h + 1) * P, :],
                                    in_=cxb)

                if plimit != 4:
                    nc.gpsimd.collective_compute(
                        "AllToAll", ALU.bypass, replica_groups=RG,
                        ins=[a2a_in[:]], outs=[a2a_out[:]])

                    # ------- Phase C2: x_mid = x_rows + ctx_rows @ Wo ---------
                    with (
                        tc.tile_pool(name="pc2", bufs=2) as pc2,
                        tc.tile_pool(name="pc21", bufs=1) as pc21,
                        tc.tile_pool(name="pc2_ps", bufs=2,
                                     space="PSUM") as pc2ps,
                        tc.tile_pool(name="pc2_ps2", bufs=2,
                                     space="PSUM") as pc2ps2,
                    ):
                        ctx_sb = pc21.tile([P, ND, RT], BF16, tag="ctxsb")
                        nc.sync.dma_start(
                            out=ctx_sb,
                            in_=a2a_out.rearrange("(c p) t -> p c t", p=P))
                        for do in range(ND):
                            op_ = pc2ps.tile([P, RT], F32, tag="op")
                            for dc in range(ND):
                                nc.tensor.matmul(
                                    out=op_[:],
                                    lhsT=wo_sb[:, dc, do * P:(do + 1) * P],
                                    rhs=ctx_sb[:, dc, :],
                                    start=(dc == 0), stop=(dc == ND - 1))
                            ot_sb = pc2.tile([P, RT], F32, tag="otsb")
                            nc.vector.tensor_copy(out=ot_sb, in_=op_)
                            for r in range(NRT):
                                tp = pc2ps2.tile([P, P], F32, tag="tp2")
                                nc.tensor.transpose(
                                    out=tp, in_=ot_sb[:, r * P:(r + 1) * P],
                                    identity=c_ident)
                                nc.vector.tensor_add(
                                    x_mid[:, r, do * P:(do + 1) * P],
                                    xr2[:, r, do * P:(do + 1) * P], tp)

                # wo_sb/xr2 no longer needed; free their SBUF before phase E
                pwo_ctx.close()

            if plimit >= 2 and plimit != 4:

                # ---------------- Phase D: residual, h, router ----------------
                with (
                    tc.tile_pool(name="pd", bufs=2) as pd,
                    tc.tile_pool(name="pd1", bufs=1) as pd1,
                    tc.tile_pool(name="pd_ps", bufs=2, space="PSUM") as pdps,
                    tc.tile_pool(name="pd_ps2", bufs=1, space="PSUM") as pdps2,
                ):
                    h_sb = pd1.tile([P, NRT, D], F32, tag="hsb")
                    hT_c = pd1.tile([P, ND, RT], F32, tag="hTc")
                    scr3 = pd1.tile([P, D], F32, tag="scr3")
                    for r in range(NRT):
                        ms = pd.tile([P, 1], F32, tag="ms")
                        nc.scalar.activation(out=scr3, in_=x_mid[:, r, :],
                                             func=ACTF.Square, accum_out=ms)
                        nc.scalar.activation(out=ms, in_=ms, func=ACTF.Sqrt,
                                             bias=c_eps, scale=1.0 / D)
                        nc.vector.reciprocal(out=ms, in_=ms)
                        nc.vector.scalar_tensor_tensor(
                            out=h_sb[:, r, :], in0=x_mid[:, r, :], scalar=ms,
                            in1=c_fnw, op0=ALU.mult, op1=ALU.mult)
                        h16 = pd.tile([P, D], BF16, tag="h16")
                        nc.vector.tensor_copy(out=h16, in_=h_sb[:, r, :])
                        nc.sync.dma_start(out=hcomb[r * P:(r + 1) * P, 0:D],
                                          in_=h16)
                        for dq in range(4):
                            tp4 = pdps.tile([P, 4, P], F32, tag="tp")
                            for k in range(4):
                                dc = dq * 4 + k
                                nc.tensor.transpose(
                                    out=tp4[:, k, :],
                                    in_=h_sb[:, r, dc * P:(dc + 1) * P],
                                    identity=c_ident)
                            nc.vector.tensor_copy(
                                out=hT_c[:, dq * 4:(dq + 1) * 4,
                                         r * P:(r + 1) * P],
                                in_=tp4)
                    # router logits (plain fp32 matmuls, exact)
                    lgp = pdps2.tile([NE, RT], F32, tag="lgp")
                    for dc in range(ND):
                        nc.tensor.matmul(out=lgp[:], lhsT=c_wgate[:, dc, :],
                                         rhs=hT_c[:, dc, :],
                                         start=(dc == 0), stop=(dc == ND - 1))
                    lg_sb = pd1.tile([NE, RT], F32, tag="lgsb")
                    nc.vector.tensor_copy(out=lg_sb, in_=lgp)
                    lg_t = pd1.tile([P, NRT, NE], F32, tag="lgt")
                    for r in range(NRT):
                        tp = pdps.tile([P, NE], F32, tag="tpl")
                        nc.tensor.transpose(out=tp, in_=lg_sb[:, r * P:(r + 1) * P],
                                            identity=c_ident[:NE, :NE])
                        nc.vector.tensor_copy(out=lg_t[:, r, :], in_=tp)
                    for r in range(NRT):
                        row = lg_t[:, r, :]
                        mx = pd.tile([P, 8], F32, tag="mx")
                        nc.vector.max(out=mx, in_=row)
                        nm1 = pd.tile([P, 1], F32, tag="nm1")
                        nc.vector.tensor_scalar_mul(nm1, mx[:, 0:1], -1.0)
                        g = pd.tile([P, NE], F32, tag="g")
                        d8 = pd.tile([P, 1], F32, tag="d8")
                        nc.scalar.activation(out=g, in_=row, func=ACTF.Exp,
                                             bias=nm1, accum_out=d8)
                        nc.vector.reciprocal(out=d8, in_=d8)
                        nc.vector.tensor_scalar_mul(g, g, d8)
                        mg = pd.tile([P, 8], F32, tag="mg")
                        nc.vector.max(out=mg, in_=g)
                        msk = pd.tile([P, NE], F32, tag="msk")
                        nc.vector.tensor_scalar(out=msk, in0=g, scalar1=mg[:, 1:2],
                                                scalar2=None, op0=ALU.is_ge)
                        comb = pd.tile([P, NE], F32, tag="comb")
                        nc.vector.tensor_mul(comb, g, msk)
                        cb16 = pd.tile([P, NE], BF16, tag="cb16")
                        nc.vector.tensor_copy(out=cb16, in_=comb)
                        nc.sync.dma_start(out=hcomb[r * P:(r + 1) * P, D:D + NE],
                                          in_=cb16)
                        if debug:
                            nc.sync.dma_start(out=comb_dbg[r * P:(r + 1) * P, :],
                                              in_=comb)
                            nc.sync.dma_start(out=lgt_dbg[r * P:(r + 1) * P, :],
                                              in_=lg_t[:, r, :])
                            nc.sync.dma_start(out=xmid_dbg[r * P:(r + 1) * P, :],
                                              in_=x_mid[:, r, :])

                nc.gpsimd.collective_compute(
                    "AllGather", ALU.bypass, replica_groups=RGP,
                    ins=[hcomb[:]], outs=[hcomb_all[:]])

            if plimit == 1:
                with tc.tile_pool(name="px1", bufs=2) as px1:
                    for r in range(NRT):
                        nc.sync.dma_start(out=out_r[r * P:(r + 1) * P, :],
                                          in_=x_mid[:, r, :])
            if plimit == 2:
                with tc.tile_pool(name="px2", bufs=2) as px2:
                    for r in range(NRT):
                        t2 = px2.tile([P, D], BF16, tag="t2")
                        nc.sync.dma_start(out=t2,
                                          in_=hcomb_all[r * P:(r + 1) * P, 0:D])
                        t2f = px2.tile([P, D], F32, tag="t2f")
                        nc.vector.tensor_copy(out=t2f, in_=t2)
                        nc.sync.dma_start(out=out_r[r * P:(r + 1) * P, :],
                                          in_=t2f)
            if plimit >= 3:
                # ------- Phase E: 4 experts x 512 pair-local tokens ----------
                with (
                    tc.tile_pool(name="pe1", bufs=1) as pe1,
                    tc.tile_pool(name="pew", bufs=3) as pew,
                    tc.tile_pool(name="pes", bufs=2) as pes,
                    tc.tile_pool(name="pes1", bufs=1) as pes1,
                    tc.tile_pool(name="pe_ps", bufs=2, space="PSUM") as peps,
                    tc.tile_pool(name="pe_ps2", bufs=2, space="PSUM") as peps2,
                    tc.tile_pool(name="pe_ps3", bufs=2, space="PSUM") as peps3,
                ):
                    c_identb = pe1.tile([P, P], BF16, tag="identb")
                    nc.sync.dma_start(out=c_identb, in_=identb[:])
                    NJ = PT // P    # 4 token tiles of 128
                    hT_g = pe1.tile([P, ND, PT], BF16, tag="hTg")
                    act_g = pe1.tile([P, NEH, PT], BF16, tag="actg")
                    combg = pe1.tile([P, NJ, EPC], F32, tag="combg")
                    yacc = pe1.tile([P, NJ, D], F32, tag="yacc")
                    for j in range(NJ):
                        tt0 = j * P
                        hb = pes1.tile([P, D], BF16, tag="hb")
                        nc.sync.dma_start(out=hb, in_=hcomb_all[tt0:tt0 + P, 0:D])
                        for dq in range(4):
                            tp4 = peps.tile([P, 4, P], BF16, tag="peab")
                            for k in range(4):
                                dc = dq * 4 + k
                                nc.tensor.transpose(
                                    out=tp4[:, k, :],
                                    in_=hb[:, dc * P:(dc + 1) * P],
                                    identity=c_identb)
                            nc.vector.tensor_copy(
                                out=hT_g[:, dq * 4:(dq + 1) * 4,
                                         j * P:(j + 1) * P],
                                in_=tp4)
                        cbl = pes.tile([P, NE], BF16, tag="cbl")
                        nc.sync.dma_start(out=cbl,
                                          in_=hcomb_all[tt0:tt0 + P, D:D + NE])
                        for ei in range(EPC):
                            cbm = pes.tile([P, NE], F32, tag="cbm")
                            nc.vector.tensor_mul(cbm, cbl, c_esel4[:, ei, :])
                            nc.vector.tensor_reduce(
                                out=combg[:, j, ei:ei + 1], in_=cbm,
                                axis=AX.X, op=ALU.add)
                    for ei in range(EPC):
                        for et in range(NEH):
                            wi_s = pew.tile([P, ND, P], BF16, tag="wis")
                            nc.sync.dma_start(out=wi_s, in_=wi_e[ei, et])
                            wg_s = pew.tile([P, ND, P], BF16, tag="wgs")
                            nc.sync.dma_start(out=wg_s, in_=wg_e[ei, et])
                            upp = peps3.tile([P, 512], F32, tag="upp")
                            gtp = peps2.tile([P, 512], F32, tag="peb")
                            for dc in range(ND):
                                nc.tensor.matmul(
                                    out=upp[:], lhsT=wi_s[:, dc, :],
                                    rhs=hT_g[:, dc, :],
                                    start=(dc == 0), stop=(dc == ND - 1))
                            for dc in range(ND):
                                nc.tensor.matmul(
                                    out=gtp[:], lhsT=wg_s[:, dc, :],
                                    rhs=hT_g[:, dc, :],
                                    start=(dc == 0), stop=(dc == ND - 1))
                            sil = pes.tile([P, 512], BF16, tag="sil")
                            nc.scalar.activation(out=sil, in_=gtp, func=ACTF.Silu)
                            nc.vector.tensor_tensor(
                                out=act_g[:, et, :], in0=sil, in1=upp,
                                op=ALU.mult)
                        for dt in range(ND):
                            wo_s = pew.tile([P, NEH, P], BF16, tag="wos")
                            nc.sync.dma_start(out=wo_s, in_=woe[ei, dt])
                            yp = peps.tile([P, 512], F32, tag="pea")
                            for ec in range(NEH):
                                nc.tensor.matmul(
                                    out=yp[:], lhsT=wo_s[:, ec, :],
                                    rhs=act_g[:, ec, :],
                                    start=(ec == 0), stop=(ec == NEH - 1))
                            ysb = pes.tile([P, 512], F32, tag="ysb")
                            nc.vector.tensor_copy(out=ysb, in_=yp)
                            for q in range(NJ):
                                tp = peps2.tile([P, 512], F32, tag="peb")
                                nc.tensor.transpose(
                                    out=tp[:, :P], in_=ysb[:, q * P:(q + 1) * P],
                                    identity=c_ident)
                                dst = yacc[:, q, dt * P:(dt + 1) * P]
                                if ei == 0:
                                    nc.vector.tensor_scalar_mul(
                                        dst, tp[:, :P], combg[:, q, 0:1])
                                else:
                                    nc.vector.scalar_tensor_tensor(
                                        out=dst, in0=tp[:, :P],
                                        scalar=combg[:, q, ei:ei + 1],
                                        in1=dst, op0=ALU.mult, op1=ALU.add)
                    for j in range(NJ):
                        y16 = pes.tile([P, D], BF16, tag="y16")
                        nc.vector.tensor_copy(out=y16, in_=yacc[:, j, :])
                        nc.sync.dma_start(out=ybuf[j * P:(j + 1) * P, :],
                                          in_=y16)

                if plimit != 4:
                    nc.gpsimd.collective_compute(
                        "ReduceScatter", ALU.add, replica_groups=RGP,
                        ins=[ybuf[:]], outs=[rs2[:]])

                # ---------------- Phase F: final residual ---------------------
                with tc.tile_pool(name="pf", bufs=2) as pf:
                    for r in range(NRT):
                        rr = pf.tile([P, D], BF16, tag="rr2")
                        src_t = ybuf if plimit == 4 else rs2
                        nc.sync.dma_start(out=rr, in_=src_t[r * P:(r + 1) * P, :])
                        ot = pf.tile([P, D], F32, tag="ot")
                        if plimit == 4:
                            nc.vector.tensor_copy(out=ot, in_=rr)
                            nc.sync.dma_start(out=out_r[r * P:(r + 1) * P, :],
                                              in_=ot)
                        else:
                            nc.vector.tensor_add(ot, x_mid[:, r, :], rr)
                            nc.sync.dma_start(out=out_r[r * P:(r + 1) * P, :],
                                              in_=ot)


    nc.finalize()
    return nc, debug


_PROG = None


def _get_prog():
    global _PROG
    if _PROG is None:
        _PROG = _build()
    return _PROG


def _rope_tables():
    inv_freq = 1.0 / (ROPE_BASE ** (np.arange(0, HD, 2, dtype=np.float32) / HD))
    t = np.arange(T, dtype=np.float32)
    freqs = np.einsum("i,j->ij", t, inv_freq).astype(np.float32)
    emb = np.concatenate((freqs, freqs), axis=-1)
    return np.cos(emb).astype(np.float32), np.sin(emb).astype(np.float32)


def _wtile_in(w):
    """[D, EH] -> [NEH, P, ND, P] bf16: contiguous per-et lhsT strips."""
    return np.ascontiguousarray(
        w.reshape(ND, P, NEH, P).transpose(2, 1, 0, 3)
    ).astype(ml_dtypes.bfloat16)


def _wtile_out(w):
    """[EH, D] -> [ND, P, NEH, P] bf16: contiguous per-dt lhsT strips."""
    return np.ascontiguousarray(
        w.reshape(NEH, P, ND, P).transpose(2, 1, 0, 3)
    ).astype(ml_dtypes.bfloat16)


_PREP_CACHE = {}


def _make_in_maps(inputs):
    x = np.ascontiguousarray(np.asarray(inputs["x"], np.float32).reshape(T, D))
    mask = np.asarray(inputs["attn_mask"], np.float32).reshape(T, T)
    causal = np.triu(np.full((T, T), NEG, np.float32), k=1)
    if not np.array_equal(mask, causal):
        raise NotImplementedError("kernel compiled for the causal attn_mask")

    Wq = np.asarray(inputs["Wq"], np.float32)
    Wk = np.asarray(inputs["Wk"], np.float32)
    Wv = np.asarray(inputs["Wv"], np.float32)
    Wo = np.asarray(inputs["Wo"], np.float32)
    wi = np.asarray(inputs["wi"], np.float32)
    wg = np.asarray(inputs["wg"], np.float32)
    wo = np.asarray(inputs["wo"], np.float32)
    cos_np, sin_np = _rope_tables()
    tri = np.triu(np.ones((P, P), np.float32))           # [k, q]: 1 if q >= k
    ident_np = np.eye(P, dtype=np.float32)

    key = (np.asarray(inputs["wi"]).ctypes.data,
           np.asarray(inputs["x"]).ctypes.data)
    cached = _PREP_CACHE.get(key)
    if cached is not None:
        return cached
    wi_all = np.stack([_wtile_in(wi[e]) for e in range(NE)])
    wg_all = np.stack([_wtile_in(wg[e]) for e in range(NE)])
    wo_all = np.stack([_wtile_out(wo[e]) for e in range(NE)])
    Wo_b16 = np.ascontiguousarray(Wo).astype(ml_dtypes.bfloat16)
    in_maps = []
    for c in range(NCORES):
        g = c // 2
        anw_col = np.asarray(inputs["attn_norm_w"],
                             np.float32).reshape(D, 1)
        wqkv_c = np.ascontiguousarray(np.concatenate(
            [Wq[:, 2 * c * HD:(2 * c + 2) * HD],
             Wk[:, g * HD:(g + 1) * HD],
             Wv[:, g * HD:(g + 1) * HD]], axis=1) * anw_col)
        e0 = EPC * (c % 2)
        esel4_c = np.zeros((EPC, NE), np.float32)
        for i in range(EPC):
            esel4_c[i, e0 + i] = 1.0
        in_maps.append({
            "x_full": x,
            "x_rows": np.ascontiguousarray(x[c * RT:(c + 1) * RT, :]),
            "wqkv": wqkv_c,
            "wo_full": Wo_b16,
            "wgate": np.ascontiguousarray(np.asarray(inputs["w_gate"],
                                                     np.float32)),
            "anw": np.asarray(inputs["attn_norm_w"], np.float32).reshape(1, D),
            "fnw": np.asarray(inputs["ffn_norm_w"], np.float32).reshape(1, D),
            "qnw": np.asarray(inputs["q_norm_w"], np.float32).reshape(1, HD),
            "knw": np.asarray(inputs["k_norm_w"], np.float32).reshape(1, HD),
            "cos_t": cos_np,
            "sin_t": sin_np,
            "tri01": tri,
            "ident": ident_np,
            "identb": ident_np.astype(ml_dtypes.bfloat16),
            "esel4": esel4_c,
            "onesr": np.ones((P, 1), np.float32),
            "wi_e": wi_all[e0:e0 + EPC],
            "wg_e": wg_all[e0:e0 + EPC],
            "woe": wo_all[e0:e0 + EPC],
        })
    return in_maps


_RUNNER = None


def _get_runner():
    """Persistent jitted SPMD executor (compiles once per process)."""
    global _RUNNER
    if _RUNNER is None:
        import jax
        from jax.experimental.shard_map import shard_map
        from jax.sharding import Mesh, PartitionSpec

        from concourse import bass2jax as b2j

        nc, debug = _get_prog()
        b2j.install_neuronx_cc_hook()
        pname = nc.partition_id_tensor.name if nc.partition_id_tensor else None
        in_names, out_names, out_avals, zero_specs = [], [], [], []
        for alloc in nc.m.functions[0].allocations:
            if not isinstance(alloc, mybir.MemoryLocationSet):
                continue
            name = alloc.memorylocations[0].name
            if alloc.kind == "ExternalInput":
                if name != pname:
                    in_names.append(name)
            elif alloc.kind == "ExternalOutput":
                out_names.append(name)
                shape = tuple(alloc.tensor_shape)
                dt_np = mybir.dt.np(alloc.dtype)
                out_avals.append(jax.core.ShapedArray(shape, dt_np))
                zero_specs.append((shape, dt_np))
        n_params = len(in_names)
        all_in = list(in_names) + list(out_names) + ([pname] if pname else [])
        donate = tuple(range(n_params, n_params + len(out_names)))

        def _body(*args):
            operands = list(args)
            if pname is not None:
                operands.append(b2j.partition_id_tensor())
            outs = b2j._bass_exec_p.bind(
                *operands, out_avals=tuple(out_avals), in_names=tuple(all_in),
                out_names=tuple(out_names), lowering_input_output_aliases=(),
                sim_require_finite=True, sim_require_nnan=True, nc=nc)
            return tuple(outs)

        devices = jax.devices()[:NCORES]
        mesh = Mesh(np.asarray(devices), ("core",))
        nio = n_params + len(out_names)
        sharded = jax.jit(
            shard_map(_body, mesh=mesh, in_specs=(PartitionSpec("core"),) * nio,
                      out_specs=(PartitionSpec("core"),) * len(out_names),
                      check_rep=False),
            donate_argnums=donate, keep_unused=True)
        _RUNNER = (sharded, in_names, out_names, zero_specs, debug)
    return _RUNNER


def _run(in_maps):
    sharded, in_names, out_names, zero_specs, debug = _get_runner()
    concat_in = [
        np.concatenate([np.asarray(in_maps[c][nm]) for c in range(NCORES)],
                       axis=0)
        for nm in in_names
    ]
    zeros = [np.zeros((NCORES * s[0],) + tuple(s[1:]), d)
             for (s, d) in zero_specs]
    outs = sharded(*concat_in, *zeros)
    return {nm: np.asarray(outs[i]) for i, nm in enumerate(out_names)}, debug


def kernel(**inputs):
    in_maps = _make_in_maps(inputs)
    res, debug = _run(in_maps)
    out = res["out_r"]  # [NCORES*RT, D] = [T, D], rank-concat = token order
    if debug:
        kernel._dbg = res
    return out.reshape(1, T, D).astype(np.float32)



# revision 58
# speedup vs baseline: 1.0425x; 1.0240x over previous
"""Trainium2 Bass kernel for nn_DecoderBlock (attention + top-2 MoE), 8 cores.

Sharding:
  - Attention: tensor-parallel over heads (2 Q heads + their KV head per
    core); per-head context is exchanged with a small bf16 AllToAll so each
    core applies the full Wo to its own 256 token rows locally (no big
    ReduceScatter of [T, D] partials).
  - Router: replicated math on each core's token rows (fp32 matmuls).
  - MoE: pair-wise sharding. Cores {2g, 2g+1} share a 512-token block;
    each core runs 4 of the 8 experts densely over the block (scaled by
    the top-2 combine weight, zero if not routed). h+comb are AllGathered
    only within the pair, and a pair ReduceScatter sums the two cores'
    expert contributions back to each core's 256 token rows. This keeps
    expert flops identical to 1-expert-per-core but shrinks the two MoE
    collectives from all-8 broadcast volume to pair-local volume.
Precision:
  - Attention matmuls run as float32r (full-speed PE mode, ~1.5e-4 rel err),
    router matmul in plain fp32, expert FFN in bf16 (weights host-cast).
  - All three collectives (attn ReduceScatter, h AllGather, expert-output
    ReduceScatter) carry bf16 payloads: collective wire time dominates the
    on-device cost, and halving the bytes keeps rel err ~1.3e-3 (<< 2e-2).
"""
import os
import sys

import numpy as np

for _p in ("/opt/trn_rl_repo", "/root/.axon_site/_ro/trn_rl_repo"):
    if os.path.isdir(_p) and _p not in sys.path:
        sys.path.append(_p)

import ml_dtypes  # noqa: E402

import concourse.bacc as bacc  # noqa: E402
import concourse.bass as bass  # noqa: E402
import concourse.tile as tile  # noqa: E402
from concourse import mybir  # noqa: E402
from concourse.bass_utils import run_bass_kernel_spmd  # noqa: E402

F32 = mybir.dt.float32
F32R = mybir.dt.float32r
BF16 = mybir.dt.bfloat16
AX = mybir.AxisListType
ALU = mybir.AluOpType
ACTF = mybir.ActivationFunctionType

T = 2048          # tokens
D = 2048          # model dim
P = 128           # partitions
NT = T // P       # 16 token tiles
ND = D // P       # 16 dim chunks
HD = 128          # head dim
NQ = 16           # query heads
NE = 8            # experts
EH = 4096         # expert hidden
NEH = EH // P     # 32
NCORES = 8
RT = T // NCORES  # 256 rows per core
NRT = RT // P     # 2
EPC = 4           # experts per core (pair-wise MoE sharding)
PT = 2 * RT       # 512 tokens per core pair
EPS = 1e-6
ROPE_BASE = 5e6
NEG = -1e9
SM_SCALE = 1.0 / float(np.sqrt(HD))
HPC = NQ // NCORES   # 2 q heads per core


def _pbcast(ap, p=P):
    """AP that broadcasts a [1, ...] source across p partitions (DMA only)."""
    return bass.AP(tensor=ap.tensor, offset=ap.offset,
                   ap=[[0, p]] + [list(x) for x in ap.ap[1:]])


def _build():
    nc = bacc.Bacc()

    dp = nc.declare_dram_parameter
    x_full = dp("x_full", [T, D], F32, isOutput=False)
    x_rows = dp("x_rows", [RT, D], F32, isOutput=False)
    wqkv = dp("wqkv", [D, 512], F32R, isOutput=False)      # [Wq 2 heads | Wk | Wv]
    wo_full = dp("wo_full", [D, D], BF16, isOutput=False)   # full Wo (bf16)
    wgate = dp("wgate", [D, NE], F32, isOutput=False)
    anw = dp("anw", [1, D], F32, isOutput=False)
    fnw = dp("fnw", [1, D], F32, isOutput=False)
    qnw = dp("qnw", [1, HD], F32, isOutput=False)
    knw = dp("knw", [1, HD], F32, isOutput=False)
    cos_t = dp("cos_t", [T, HD], F32, isOutput=False)
    sin_t = dp("sin_t", [T, HD], F32, isOutput=False)
    tri01 = dp("tri01", [P, P], F32, isOutput=False)
    ident = dp("ident", [P, P], F32, isOutput=False)
    identb = dp("identb", [P, P], BF16, isOutput=False)
    esel4 = dp("esel4", [EPC, NE], F32, isOutput=False)
    onesr = dp("onesr", [P, 1], F32R, isOutput=False)
    wi_e = dp("wi_e", [EPC, NEH, P, ND, P], BF16, isOutput=False)
    wg_e = dp("wg_e", [EPC, NEH, P, ND, P], BF16, isOutput=False)
    woe = dp("woe", [EPC, ND, P, NEH, P], BF16, isOutput=False)

    out_r = dp("out_r", [RT, D], F32, isOutput=True)
    debug = bool(int(os.environ.get("DECODER_DEBUG", "0")))
    plimit = int(os.environ.get("DECODER_PHASE_LIMIT", "3"))
    if debug:
        xmid_dbg = dp("xmid_dbg", [RT, D], F32, isOutput=True)
        comb_dbg = dp("comb_dbg", [RT, NE], F32, isOutput=True)
        lgt_dbg = dp("lgt_dbg", [RT, NE], F32, isOutput=True)

    a2a_in = nc.dram_tensor("a2a_in", [T, RT], BF16)
    a2a_out = nc.dram_tensor("a2a_out", [T, RT], BF16)
    hcombT = nc.dram_tensor("hcombT", [D, RT], BF16)
    hcombT_all = nc.dram_tensor("hcombT_all", [2 * D, RT], BF16)
    combB = nc.dram_tensor("combB", [RT, NE], BF16)
    comb_all = nc.dram_tensor("comb_all", [PT, NE], BF16)
    ybuf = nc.dram_tensor("ybuf", [PT, D], BF16)
    rs2 = nc.dram_tensor("rs2", [RT, D], BF16)
    RG = [list(range(NCORES))]
    RGP = [[2 * g, 2 * g + 1] for g in range(NCORES // 2)]

    repeat = int(os.environ.get("DECODER_REPEAT", "1"))
    hwloop = int(os.environ.get("DECODER_HWLOOP", "0"))
    trace_sim = bool(int(os.environ.get("DECODER_TRACE_SIM", "0")))
    from contextlib import nullcontext

    with tile.TileContext(nc, trace_sim=trace_sim) as tc:
      with (tc.For_i(0, hwloop, 1) if hwloop else nullcontext()):
       for _rep in range(repeat):
        with (
            tc.tile_pool(name=f"consts{_rep}", bufs=1) as cp,
            tc.tile_pool(name=f"xmid{_rep}", bufs=1) as xp,
        ):
            c_ident = cp.tile([P, P], F32, tag="ident")
            nc.sync.dma_start(out=c_ident, in_=ident[:])
            c_tri = cp.tile([P, P], F32, tag="tri")
            nc.sync.dma_start(out=c_tri, in_=tri01[:])
            c_anw = cp.tile([P, D], F32, tag="anw")
            nc.gpsimd.dma_start(out=c_anw, in_=_pbcast(anw[:]))
            c_fnw = cp.tile([P, D], F32, tag="fnw")
            nc.gpsimd.dma_start(out=c_fnw, in_=_pbcast(fnw[:]))
            c_qnw = cp.tile([P, HD], F32, tag="qnw")
            nc.gpsimd.dma_start(out=c_qnw, in_=_pbcast(qnw[:]))
            c_knw = cp.tile([P, HD], F32, tag="knw")
            nc.gpsimd.dma_start(out=c_knw, in_=_pbcast(knw[:]))
            c_esel4 = cp.tile([P, EPC, NE], F32, tag="esel4")
            for _i in range(EPC):
                nc.gpsimd.dma_start(out=c_esel4[:, _i, :],
                                    in_=_pbcast(esel4[_i:_i + 1, :]))
            c_wgate = cp.tile([P, ND, NE], F32, tag="wgate")
            nc.sync.dma_start(out=c_wgate,
                              in_=wgate.rearrange("(c p) e -> p c e", p=P))
            c_ones = cp.tile([P, 1], F32R, tag="ones")
            nc.sync.dma_start(out=c_ones, in_=onesr[:])
            c_eps = cp.tile([P, 1], F32, tag="eps")
            nc.vector.memset(c_eps, EPS)
            c_ones1 = cp.tile([1, P], F32, tag="ones1")
            nc.vector.memset(c_ones1, 1.0)

            x_mid = xp.tile([P, NRT, D], F32, tag="xmid")
            from contextlib import ExitStack
            pwo_ctx = ExitStack()

            # qT/kT/vv/ctxT survive phases A..C
            if plimit == 4:
                pass
            else:
             with tc.tile_pool(name="qkv_keep", bufs=1) as pk:
                qT = pk.tile([P, HPC, T], F32R, tag="qT")    # [hd, head, tok]
                kT = pk.tile([P, T], F32R, tag="kT")         # [hd, tok]
                vv = pk.tile([P, NT, HD], F32R, tag="vv")    # [tok, kt, hd]
                ctxT = pk.tile([P, HPC, T], F32R, tag="ctxT")

                # ---------------- Phase A: rmsnorm + QKV projection ----------
                with (
                    tc.tile_pool(name="pa2", bufs=2) as pa2,
                    tc.tile_pool(name="pa1", bufs=1) as pa1,
                    tc.tile_pool(name="pas", bufs=2) as pas,
                    tc.tile_pool(name="pa_ps", bufs=2, space="PSUM") as paps,
                    tc.tile_pool(name="pa_ps2", bufs=3, space="PSUM") as paps2,
                ):
                    c_cos = pa1.tile([P, NT, HD], F32, tag="cos")
                    nc.sync.dma_start(out=c_cos,
                                      in_=cos_t.rearrange("(t p) d -> p t d", p=P))
                    c_sin = pa1.tile([P, NT, HD], F32, tag="sin")
                    nc.sync.dma_start(out=c_sin,
                                      in_=sin_t.rearrange("(t p) d -> p t d", p=P))
                    w_qkv = pa1.tile([P, ND, 512], F32R, tag="wqkv")
                    nc.sync.dma_start(out=w_qkv,
                                      in_=wqkv.rearrange("(c p) n -> p c n", p=P))
                    scr = pa1.tile([P, D], F32, tag="scr")

                    def _at_chain(tt):
                        # rmsnorm-scaled row tile; issued one tile ahead, and
                        # applied on the ACT engine (attn_norm_w is folded into
                        # the QKV weights host-side) so the wide apply doesn't
                        # clog the in-order DVE queue.
                        xt = pa2.tile([P, D], F32, tag="xt")
                        nc.sync.dma_start(out=xt,
                                          in_=x_full[tt * P:(tt + 1) * P, :])
                        ms = pas.tile([P, 1], F32, tag="ms")
                        nc.scalar.activation(out=scr, in_=xt, func=ACTF.Square,
                                             accum_out=ms)
                        nc.scalar.activation(out=ms, in_=ms, func=ACTF.Sqrt,
                                             bias=c_eps, scale=1.0 / D)
                        nc.vector.reciprocal(out=ms, in_=ms)
                        at = pa2.tile([P, D], F32, tag="at")
                        nc.scalar.activation(out=at, in_=xt, func=ACTF.Copy,
                                             scale=ms)
                        return at

                    at_cur = _at_chain(0)
                    for tt in range(NT):
                        aT = pa1.tile([P, ND, P], F32R, tag="aT")
                        for dq in range(4):
                            tp4 = paps.tile([P, 4, P], F32, tag="tp")
                            for k in range(4):
                                dc = dq * 4 + k
                                nc.tensor.transpose(
                                    out=tp4[:, k, :],
                                    in_=at_cur[:, dc * P:(dc + 1) * P],
                                    identity=c_ident)
                            nc.vector.tensor_copy(
                                out=aT[:, dq * 4:(dq + 1) * 4, :], in_=tp4)
                        qkvp = paps2.tile([P, 512], F32, tag="qkvp")
                        for dc in range(ND):
                            nc.tensor.matmul(out=qkvp[:],
                                             lhsT=aT[:, dc, :],
                                             rhs=w_qkv[:, dc, :],
                                             start=(dc == 0), stop=(dc == ND - 1))
                        if tt + 1 < NT:
                            at_next = _at_chain(tt + 1)
                        # q heads + k: per-head rmsnorm + rope, then transpose
                        for ih in range(HPC + 1):
                            seg = qkvp[:, ih * HD:(ih + 1) * HD]
                            wnorm = c_qnw if ih < HPC else c_knw
                            scr2 = pas.tile([P, HD], F32, tag="scr2")
                            ms2 = pas.tile([P, 1], F32, tag="ms2")
                            nc.scalar.activation(out=scr2, in_=seg,
                                                 func=ACTF.Square, accum_out=ms2)
                            nc.scalar.activation(out=ms2, in_=ms2,
                                                 func=ACTF.Sqrt,
                                                 bias=c_eps, scale=1.0 / HD)
                            nc.vector.reciprocal(out=ms2, in_=ms2)
                            nrm = pas.tile([P, HD], F32, tag="nrm")
                            nc.vector.scalar_tensor_tensor(
                                out=nrm, in0=seg, scalar=ms2, in1=wnorm,
                                op0=ALU.mult, op1=ALU.mult)
                            rop = pas.tile([P, HD], F32, tag="rop")
                            nc.vector.tensor_scalar_mul(
                                rop[:, :HD // 2], nrm[:, HD // 2:], -1.0)
                            nc.vector.tensor_copy(
                                out=rop[:, HD // 2:], in_=nrm[:, :HD // 2])
                            nc.vector.tensor_mul(nrm, nrm, c_cos[:, tt, :])
                            nc.vector.tensor_mul(rop, rop, c_sin[:, tt, :])
                            nc.vector.tensor_add(nrm, nrm, rop)
                            tp2 = paps.tile([P, P], F32, tag="tp")
                            nc.tensor.transpose(out=tp2, in_=nrm, identity=c_ident)
                            dst = (qT[:, ih, tt * P:(tt + 1) * P] if ih < HPC
                                   else kT[:, tt * P:(tt + 1) * P])
                            nc.vector.tensor_copy(out=dst, in_=tp2)
                        nc.vector.tensor_copy(out=vv[:, tt, :], in_=qkvp[:, 384:512])
                        if tt + 1 < NT:
                            at_cur = at_next

                # prefetch phase-C2 operands while attention runs
                # (SBUF for these frees up when the phase-A pools close)
                if plimit != 4:
                    pwo = pwo_ctx.enter_context(
                        tc.tile_pool(name=f"pwo{_rep}", bufs=1))
                    wo_sb = pwo.tile([P, ND, D], BF16, tag="wosb")
                    nc.sync.dma_start(
                        out=wo_sb,
                        in_=wo_full.rearrange("(c p) o -> p c o", p=P))
                    xr2 = pwo.tile([P, NRT, D], F32, tag="xr2")
                    nc.sync.dma_start(
                        out=xr2, in_=x_rows.rearrange("(r p) d -> p r d", p=P))

                # ---------------- Phase B: attention ----------------------
                with (
                    tc.tile_pool(name="pb", bufs=3) as pb,
                    tc.tile_pool(name="pb2", bufs=2) as pb2,
                    tc.tile_pool(name="pb_ps", bufs=2, space="PSUM") as pbps,
                    tc.tile_pool(name="pb_ps2", bufs=2, space="PSUM") as pbps2,
                    tc.tile_pool(name="pb_ps3", bufs=1, space="PSUM") as pbps3,
                ):
                    for h in range(HPC):
                        for qc in range(4):
                            cs = qc * 512
                            ctxp = pbps2.tile([P, 512], F32, tag="ctx")
                            denp = pbps3.tile([1, 512], F32, tag="den")
                            nkt = 4 * (qc + 1)
                            for kt in range(nkt):
                                lo = max(0, kt * P - cs)
                                width = 512 - lo
                                scp = pbps.tile([P, 512], F32, tag="sc")
                                nc.tensor.matmul(
                                    out=scp[:, :width],
                                    lhsT=kT[:, kt * P:(kt + 1) * P],
                                    rhs=qT[:, h, cs + lo:cs + 512],
                                    start=True, stop=True)
                                ex = pb.tile([P, 512], F32R, tag="ex")
                                nc.scalar.activation(out=ex[:, :width],
                                                     in_=scp[:, :width],
                                                     func=ACTF.Exp, scale=SM_SCALE)
                                if kt * P >= cs:
                                    # diagonal block: first 128 cols of suffix
                                    nc.vector.tensor_mul(ex[:, :P], ex[:, :P],
                                                         c_tri)
                                nc.tensor.matmul(
                                    out=ctxp[:, lo:],
                                    lhsT=vv[:, kt, :],
                                    rhs=ex[:, :width],
                                    start=(kt == 0), stop=(kt == nkt - 1))
                                nc.tensor.matmul(
                                    out=denp[:, lo:], lhsT=c_ones,
                                    rhs=ex[:, :width],
                                    start=(kt == 0), stop=(kt == nkt - 1))
                            dsb = pb2.tile([1, 512], F32, tag="dsb")
                            nc.vector.reciprocal(out=dsb, in_=denp)
                            dbc = pbps3.tile([P, 512], F32, tag="dbc")
                            nc.tensor.matmul(out=dbc[:], lhsT=c_ones1, rhs=dsb,
                                             start=True, stop=True)
                            dbc_sb = pb2.tile([P, 512], F32, tag="dbcsb")
                            nc.scalar.copy(out=dbc_sb, in_=dbc)
                            nc.vector.tensor_mul(ctxT[:, h, cs:cs + 512],
                                                 ctxp, dbc_sb)
                            # stream ctx^T out for the all-to-all as soon as
                            # this 512-token chunk of the head is final
                            for j2 in range(2):
                                j = qc * 2 + j2
                                cxb = pb.tile([P, RT], BF16, tag="cxb")
                                nc.vector.tensor_copy(
                                    out=cxb,
                                    in_=ctxT[:, h, j * RT:(j + 1) * RT])
                                nc.sync.dma_start(
                                    out=a2a_in[j * RT + h * P:
                                               j * RT + (h + 1) * P, :],
                                    in_=cxb)

                if plimit != 4:
                    nc.gpsimd.collective_compute(
                        "AllToAll", ALU.bypass, replica_groups=RG,
                        ins=[a2a_in[:]], outs=[a2a_out[:]])

                    # ------- Phase C2: x_mid = x_rows + ctx_rows @ Wo ---------
                    with (
                        tc.tile_pool(name="pc2", bufs=2) as pc2,
                        tc.tile_pool(name="pc21", bufs=1) as pc21,
                        tc.tile_pool(name="pc2_ps", bufs=2,
                                     space="PSUM") as pc2ps,
                        tc.tile_pool(name="pc2_ps2", bufs=2,
                                     space="PSUM") as pc2ps2,
                    ):
                        ctx_sb = pc21.tile([P, ND, RT], BF16, tag="ctxsb")
                        nc.sync.dma_start(
                            out=ctx_sb,
                            in_=a2a_out.rearrange("(c p) t -> p c t", p=P))
                        for do in range(ND):
                            op_ = pc2ps.tile([P, RT], F32, tag="op")
                            for dc in range(ND):
                                nc.tensor.matmul(
                                    out=op_[:],
                                    lhsT=wo_sb[:, dc, do * P:(do + 1) * P],
                                    rhs=ctx_sb[:, dc, :],
                                    start=(dc == 0), stop=(dc == ND - 1))
                            ot_sb = pc2.tile([P, RT], F32, tag="otsb")
                            nc.vector.tensor_copy(out=ot_sb, in_=op_)
                            for r in range(NRT):
                                tp = pc2ps2.tile([P, P], F32, tag="tp2")
                                nc.tensor.transpose(
                                    out=tp, in_=ot_sb[:, r * P:(r + 1) * P],
                                    identity=c_ident)
                                nc.vector.tensor_add(
                                    x_mid[:, r, do * P:(do + 1) * P],
                                    xr2[:, r, do * P:(do + 1) * P], tp)

                # wo_sb/xr2 no longer needed; free their SBUF before phase E
                pwo_ctx.close()

            if plimit >= 2 and plimit != 4:

                # ---------------- Phase D: residual, h, router ----------------
                with (
                    tc.tile_pool(name="pd", bufs=2) as pd,
                    tc.tile_pool(name="pd1", bufs=1) as pd1,
                    tc.tile_pool(name="pd_ps", bufs=2, space="PSUM") as pdps,
                    tc.tile_pool(name="pd_ps2", bufs=1, space="PSUM") as pdps2,
                ):
                    h_sb = pd1.tile([P, NRT, D], F32, tag="hsb")
                    hT_c = pd1.tile([P, ND, RT], F32, tag="hTc")
                    scr3 = pd1.tile([P, D], F32, tag="scr3")
                    for r in range(NRT):
                        ms = pd.tile([P, 1], F32, tag="ms")
                        nc.scalar.activation(out=scr3, in_=x_mid[:, r, :],
                                             func=ACTF.Square, accum_out=ms)
                        nc.scalar.activation(out=ms, in_=ms, func=ACTF.Sqrt,
                                             bias=c_eps, scale=1.0 / D)
                        nc.vector.reciprocal(out=ms, in_=ms)
                        nc.vector.scalar_tensor_tensor(
                            out=h_sb[:, r, :], in0=x_mid[:, r, :], scalar=ms,
                            in1=c_fnw, op0=ALU.mult, op1=ALU.mult)
                        for dq in range(4):
                            tp4 = pdps.tile([P, 4, P], F32, tag="tp")
                            for k in range(4):
                                dc = dq * 4 + k
                                nc.tensor.transpose(
                                    out=tp4[:, k, :],
                                    in_=h_sb[:, r, dc * P:(dc + 1) * P],
                                    identity=c_ident)
                            nc.vector.tensor_copy(
                                out=hT_c[:, dq * 4:(dq + 1) * 4,
                                         r * P:(r + 1) * P],
                                in_=tp4)
                        h16T = pd.tile([P, ND, P], BF16, tag="h16T")
                        nc.vector.tensor_copy(out=h16T,
                                              in_=hT_c[:, :, r * P:(r + 1) * P])
                        nc.sync.dma_start(
                            out=hcombT.rearrange(
                                "(c p) t -> p c t", p=P)[:, :,
                                                         r * P:(r + 1) * P],
                            in_=h16T)
                    # kick the big h^T AllGather before the router math; the
                    # tiny comb AllGather below only has to land ~600us later
                    # (at the yacc-scaling stage of phase E)
                    nc.gpsimd.collective_compute(
                        "AllGather", ALU.bypass, replica_groups=RGP,
                        ins=[hcombT[:]], outs=[hcombT_all[:]])
                    # router logits (plain fp32 matmuls, exact)
                    lgp = pdps2.tile([NE, RT], F32, tag="lgp")
                    for dc in range(ND):
                        nc.tensor.matmul(out=lgp[:], lhsT=c_wgate[:, dc, :],
                                         rhs=hT_c[:, dc, :],
                                         start=(dc == 0), stop=(dc == ND - 1))
                    lg_sb = pd1.tile([NE, RT], F32, tag="lgsb")
                    nc.vector.tensor_copy(out=lg_sb, in_=lgp)
                    lg_t = pd1.tile([P, NRT, NE], F32, tag="lgt")
                    for r in range(NRT):
                        tp = pdps.tile([P, NE], F32, tag="tpl")
                        nc.tensor.transpose(out=tp, in_=lg_sb[:, r * P:(r + 1) * P],
                                            identity=c_ident[:NE, :NE])
                        nc.vector.tensor_copy(out=lg_t[:, r, :], in_=tp)
                    for r in range(NRT):
                        row = lg_t[:, r, :]
                        mx = pd.tile([P, 8], F32, tag="mx")
                        nc.vector.max(out=mx, in_=row)
                        nm1 = pd.tile([P, 1], F32, tag="nm1")
                        nc.vector.tensor_scalar_mul(nm1, mx[:, 0:1], -1.0)
                        g = pd.tile([P, NE], F32, tag="g")
                        d8 = pd.tile([P, 1], F32, tag="d8")
                        nc.scalar.activation(out=g, in_=row, func=ACTF.Exp,
                                             bias=nm1, accum_out=d8)
                        nc.vector.reciprocal(out=d8, in_=d8)
                        nc.vector.tensor_scalar_mul(g, g, d8)
                        mg = pd.tile([P, 8], F32, tag="mg")
                        nc.vector.max(out=mg, in_=g)
                        msk = pd.tile([P, NE], F32, tag="msk")
                        nc.vector.tensor_scalar(out=msk, in0=g, scalar1=mg[:, 1:2],
                                                scalar2=None, op0=ALU.is_ge)
                        comb = pd.tile([P, NE], F32, tag="comb")
                        nc.vector.tensor_mul(comb, g, msk)
                        cb16 = pd.tile([P, NE], BF16, tag="cb16")
                        nc.vector.tensor_copy(out=cb16, in_=comb)
                        nc.sync.dma_start(out=combB[r * P:(r + 1) * P, :],
                                          in_=cb16)
                        if debug:
                            nc.sync.dma_start(out=comb_dbg[r * P:(r + 1) * P, :],
                                              in_=comb)
                            nc.sync.dma_start(out=lgt_dbg[r * P:(r + 1) * P, :],
                                              in_=lg_t[:, r, :])
                            nc.sync.dma_start(out=xmid_dbg[r * P:(r + 1) * P, :],
                                              in_=x_mid[:, r, :])

                nc.gpsimd.collective_compute(
                    "AllGather", ALU.bypass, replica_groups=RGP,
                    ins=[combB[:]], outs=[comb_all[:]])

            if plimit in (1, 2):
                with tc.tile_pool(name="px1", bufs=2) as px1:
                    for r in range(NRT):
                        nc.sync.dma_start(out=out_r[r * P:(r + 1) * P, :],
                                          in_=x_mid[:, r, :])
            if plimit >= 3:
                # ------- Phase E: 4 experts x 512 pair-local tokens ----------
                with (
                    tc.tile_pool(name="pe1", bufs=1) as pe1,
                    tc.tile_pool(name="pew", bufs=3) as pew,
                    tc.tile_pool(name="pes", bufs=2) as pes,
                    tc.tile_pool(name="pes1", bufs=1) as pes1,
                    tc.tile_pool(name="pe_ps", bufs=2, space="PSUM") as peps,
                    tc.tile_pool(name="pe_ps2", bufs=2, space="PSUM") as peps2,
                    tc.tile_pool(name="pe_ps3", bufs=2, space="PSUM") as peps3,
                ):
                    NJ = PT // P    # 4 token tiles of 128
                    hT_g = pe1.tile([P, ND, PT], BF16, tag="hTg")
                    act_g = pe1.tile([P, NEH, PT], BF16, tag="actg")
                    combg = pe1.tile([P, NJ, EPC], F32, tag="combg")
                    yacc = pe1.tile([P, NJ, D], F32, tag="yacc")
                    for half in range(2):
                        # shard `half` of the pair AG = that core's 256 rows,
                        # already transposed: load straight into the matmul rhs
                        nc.sync.dma_start(
                            out=hT_g[:, :, half * RT:(half + 1) * RT],
                            in_=hcombT_all.rearrange(
                                "(s c p) t -> s p c t", s=2, p=P)[half])
                    for j in range(NJ):
                        tt0 = j * P
                        cbl = pes.tile([P, NE], BF16, tag="cbl")
                        nc.sync.dma_start(out=cbl,
                                          in_=comb_all[tt0:tt0 + P, :])
                        for ei in range(EPC):
                            cbm = pes.tile([P, NE], F32, tag="cbm")
                            nc.vector.tensor_mul(cbm, cbl, c_esel4[:, ei, :])
                            nc.vector.tensor_reduce(
                                out=combg[:, j, ei:ei + 1], in_=cbm,
                                axis=AX.X, op=ALU.add)
                    for ei in range(EPC):
                        for et in range(NEH):
                            wi_s = pew.tile([P, ND, P], BF16, tag="wis")
                            nc.sync.dma_start(out=wi_s, in_=wi_e[ei, et])
                            wg_s = pew.tile([P, ND, P], BF16, tag="wgs")
                            nc.sync.dma_start(out=wg_s, in_=wg_e[ei, et])
                            upp = peps3.tile([P, 512], F32, tag="upp")
                            gtp = peps2.tile([P, 512], F32, tag="peb")
                            for dc in range(ND):
                                nc.tensor.matmul(
                                    out=upp[:], lhsT=wi_s[:, dc, :],
                                    rhs=hT_g[:, dc, :],
                                    start=(dc == 0), stop=(dc == ND - 1))
                            for dc in range(ND):
                                nc.tensor.matmul(
                                    out=gtp[:], lhsT=wg_s[:, dc, :],
                                    rhs=hT_g[:, dc, :],
                                    start=(dc == 0), stop=(dc == ND - 1))
                            sil = pes.tile([P, 512], BF16, tag="sil")
                            nc.scalar.activation(out=sil, in_=gtp, func=ACTF.Silu)
                            nc.vector.tensor_tensor(
                                out=act_g[:, et, :], in0=sil, in1=upp,
                                op=ALU.mult)
                        for dt in range(ND):
                            wo_s = pew.tile([P, NEH, P], BF16, tag="wos")
                            nc.sync.dma_start(out=wo_s, in_=woe[ei, dt])
                            yp = peps.tile([P, 512], F32, tag="pea")
                            for ec in range(NEH):
                                nc.tensor.matmul(
                                    out=yp[:], lhsT=wo_s[:, ec, :],
                                    rhs=act_g[:, ec, :],
                                    start=(ec == 0), stop=(ec == NEH - 1))
                            ysb = pes.tile([P, 512], F32, tag="ysb")
                            nc.vector.tensor_copy(out=ysb, in_=yp)
                            for q in range(NJ):
                                tp = peps2.tile([P, 512], F32, tag="peb")
                                nc.tensor.transpose(
                                    out=tp[:, :P], in_=ysb[:, q * P:(q + 1) * P],
                                    identity=c_ident)
                                dst = yacc[:, q, dt * P:(dt + 1) * P]
                                if ei == 0:
                                    nc.vector.tensor_scalar_mul(
                                        dst, tp[:, :P], combg[:, q, 0:1])
                                else:
                                    nc.vector.scalar_tensor_tensor(
                                        out=dst, in0=tp[:, :P],
                                        scalar=combg[:, q, ei:ei + 1],
                                        in1=dst, op0=ALU.mult, op1=ALU.add)
                    for j in range(NJ):
                        y16 = pes.tile([P, D], BF16, tag="y16")
                        nc.vector.tensor_copy(out=y16, in_=yacc[:, j, :])
                        nc.sync.dma_start(out=ybuf[j * P:(j + 1) * P, :],
                                          in_=y16)

                if plimit != 4:
                    nc.gpsimd.collective_compute(
                        "ReduceScatter", ALU.add, replica_groups=RGP,
                        ins=[ybuf[:]], outs=[rs2[:]])

                # ---------------- Phase F: final residual ---------------------
                with tc.tile_pool(name="pf", bufs=2) as pf:
                    for r in range(NRT):
                        rr = pf.tile([P, D], BF16, tag="rr2")
                        src_t = ybuf if plimit == 4 else rs2
                        nc.sync.dma_start(out=rr, in_=src_t[r * P:(r + 1) * P, :])
                        ot = pf.tile([P, D], F32, tag="ot")
                        if plimit == 4:
                            nc.vector.tensor_copy(out=ot, in_=rr)
                            nc.sync.dma_start(out=out_r[r * P:(r + 1) * P, :],
                                              in_=ot)
                        else:
                            nc.vector.tensor_add(ot, x_mid[:, r, :], rr)
                            nc.sync.dma_start(out=out_r[r * P:(r + 1) * P, :],
                                              in_=ot)


    nc.finalize()
    return nc, debug


_PROG = None


def _get_prog():
    global _PROG
    if _PROG is None:
        _PROG = _build()
    return _PROG


def _rope_tables():
    inv_freq = 1.0 / (ROPE_BASE ** (np.arange(0, HD, 2, dtype=np.float32) / HD))
    t = np.arange(T, dtype=np.float32)
    freqs = np.einsum("i,j->ij", t, inv_freq).astype(np.float32)
    emb = np.concatenate((freqs, freqs), axis=-1)
    return np.cos(emb).astype(np.float32), np.sin(emb).astype(np.float32)


def _wtile_in(w):
    """[D, EH] -> [NEH, P, ND, P] bf16: contiguous per-et lhsT strips."""
    return np.ascontiguousarray(
        w.reshape(ND, P, NEH, P).transpose(2, 1, 0, 3)
    ).astype(ml_dtypes.bfloat16)


def _wtile_out(w):
    """[EH, D] -> [ND, P, NEH, P] bf16: contiguous per-dt lhsT strips."""
    return np.ascontiguousarray(
        w.reshape(NEH, P, ND, P).transpose(2, 1, 0, 3)
    ).astype(ml_dtypes.bfloat16)


_PREP_CACHE = {}


def _make_in_maps(inputs):
    x = np.ascontiguousarray(np.asarray(inputs["x"], np.float32).reshape(T, D))
    mask = np.asarray(inputs["attn_mask"], np.float32).reshape(T, T)
    causal = np.triu(np.full((T, T), NEG, np.float32), k=1)
    if not np.array_equal(mask, causal):
        raise NotImplementedError("kernel compiled for the causal attn_mask")

    Wq = np.asarray(inputs["Wq"], np.float32)
    Wk = np.asarray(inputs["Wk"], np.float32)
    Wv = np.asarray(inputs["Wv"], np.float32)
    Wo = np.asarray(inputs["Wo"], np.float32)
    wi = np.asarray(inputs["wi"], np.float32)
    wg = np.asarray(inputs["wg"], np.float32)
    wo = np.asarray(inputs["wo"], np.float32)
    cos_np, sin_np = _rope_tables()
    tri = np.triu(np.ones((P, P), np.float32))           # [k, q]: 1 if q >= k
    ident_np = np.eye(P, dtype=np.float32)

    key = (np.asarray(inputs["wi"]).ctypes.data,
           np.asarray(inputs["x"]).ctypes.data)
    cached = _PREP_CACHE.get(key)
    if cached is not None:
        return cached
    wi_all = np.stack([_wtile_in(wi[e]) for e in range(NE)])
    wg_all = np.stack([_wtile_in(wg[e]) for e in range(NE)])
    wo_all = np.stack([_wtile_out(wo[e]) for e in range(NE)])
    Wo_b16 = np.ascontiguousarray(Wo).astype(ml_dtypes.bfloat16)
    in_maps = []
    for c in range(NCORES):
        g = c // 2
        anw_col = np.asarray(inputs["attn_norm_w"],
                             np.float32).reshape(D, 1)
        wqkv_c = np.ascontiguousarray(np.concatenate(
            [Wq[:, 2 * c * HD:(2 * c + 2) * HD],
             Wk[:, g * HD:(g + 1) * HD],
             Wv[:, g * HD:(g + 1) * HD]], axis=1) * anw_col)
        e0 = EPC * (c % 2)
        esel4_c = np.zeros((EPC, NE), np.float32)
        for i in range(EPC):
            esel4_c[i, e0 + i] = 1.0
        in_maps.append({
            "x_full": x,
            "x_rows": np.ascontiguousarray(x[c * RT:(c + 1) * RT, :]),
            "wqkv": wqkv_c,
            "wo_full": Wo_b16,
            "wgate": np.ascontiguousarray(np.asarray(inputs["w_gate"],
                                                     np.float32)),
            "anw": np.asarray(inputs["attn_norm_w"], np.float32).reshape(1, D),
            "fnw": np.asarray(inputs["ffn_norm_w"], np.float32).reshape(1, D),
            "qnw": np.asarray(inputs["q_norm_w"], np.float32).reshape(1, HD),
            "knw": np.asarray(inputs["k_norm_w"], np.float32).reshape(1, HD),
            "cos_t": cos_np,
            "sin_t": sin_np,
            "tri01": tri,
            "ident": ident_np,
            "identb": ident_np.astype(ml_dtypes.bfloat16),
            "esel4": esel4_c,
            "onesr": np.ones((P, 1), np.float32),
            "wi_e": wi_all[e0:e0 + EPC],
            "wg_e": wg_all[e0:e0 + EPC],
            "woe": wo_all[e0:e0 + EPC],
        })
    return in_maps


_RUNNER = None


def _get_runner():
    """Persistent jitted SPMD executor (compiles once per process)."""
    global _RUNNER
    if _RUNNER is None:
        import jax
        from jax.experimental.shard_map import shard_map
        from jax.sharding import Mesh, PartitionSpec

        from concourse import bass2jax as b2j

        nc, debug = _get_prog()
        b2j.install_neuronx_cc_hook()
        pname = nc.partition_id_tensor.name if nc.partition_id_tensor else None
        in_names, out_names, out_avals, zero_specs = [], [], [], []
        for alloc in nc.m.functions[0].allocations:
            if not isinstance(alloc, mybir.MemoryLocationSet):
                continue
            name = alloc.memorylocations[0].name
            if alloc.kind == "ExternalInput":
                if name != pname:
                    in_names.append(name)
            elif alloc.kind == "ExternalOutput":
                out_names.append(name)
                shape = tuple(alloc.tensor_shape)
                dt_np = mybir.dt.np(alloc.dtype)
                out_avals.append(jax.core.ShapedArray(shape, dt_np))
                zero_specs.append((shape, dt_np))
        n_params = len(in_names)
        all_in = list(in_names) + list(out_names) + ([pname] if pname else [])
        donate = tuple(range(n_params, n_params + len(out_names)))

        def _body(*args):
            operands = list(args)
            if pname is not None:
                operands.append(b2j.partition_id_tensor())
            outs = b2j._bass_exec_p.bind(
                *operands, out_avals=tuple(out_avals), in_names=tuple(all_in),
                out_names=tuple(out_names), lowering_input_output_aliases=(),
                sim_require_finite=True, sim_require_nnan=True, nc=nc)
            return tuple(outs)

        devices = jax.devices()[:NCORES]
        mesh = Mesh(np.asarray(devices), ("core",))
        nio = n_params + len(out_names)
        sharded = jax.jit(
            shard_map(_body, mesh=mesh, in_specs=(PartitionSpec("core"),) * nio,
                      out_specs=(PartitionSpec("core"),) * len(out_names),
                      check_rep=False),
            donate_argnums=donate, keep_unused=True)
        _RUNNER = (sharded, in_names, out_names, zero_specs, debug)
    return _RUNNER


def _run(in_maps):
    sharded, in_names, out_names, zero_specs, debug = _get_runner()
    concat_in = [
        np.concatenate([np.asarray(in_maps[c][nm]) for c in range(NCORES)],
                       axis=0)
        for nm in in_names
    ]
    zeros = [np.zeros((NCORES * s[0],) + tuple(s[1:]), d)
             for (s, d) in zero_specs]
    outs = sharded(*concat_in, *zeros)
    return {nm: np.asarray(outs[i]) for i, nm in enumerate(out_names)}, debug


def kernel(**inputs):
    in_maps = _make_in_maps(inputs)
    res, debug = _run(in_maps)
    out = res["out_r"]  # [NCORES*RT, D] = [T, D], rank-concat = token order
    if debug:
        kernel._dbg = res
    return out.reshape(1, T, D).astype(np.float32)



# revision 61
# speedup vs baseline: 2.5113x; 2.4089x over previous
"""Trainium2 Bass kernel for nn_DecoderBlock (attention + top-2 MoE), 8 cores.

Sharding:
  - Attention: tensor-parallel over heads (2 Q heads + their KV head per
    core); per-head context is exchanged with a small bf16 AllToAll so each
    core applies the full Wo to its own 256 token rows locally (no big
    ReduceScatter of [T, D] partials).
  - Router: replicated math on each core's token rows (fp32 matmuls).
  - MoE: pair-wise sharding. Cores {2g, 2g+1} share a 512-token block;
    each core runs 4 of the 8 experts densely over the block (scaled by
    the top-2 combine weight, zero if not routed). h+comb are AllGathered
    only within the pair, and a pair ReduceScatter sums the two cores'
    expert contributions back to each core's 256 token rows. This keeps
    expert flops identical to 1-expert-per-core but shrinks the two MoE
    collectives from all-8 broadcast volume to pair-local volume.
Precision:
  - Attention matmuls run as float32r (full-speed PE mode, ~1.5e-4 rel err),
    router matmul in plain fp32, expert FFN in bf16 (weights host-cast).
  - All three collectives (attn ReduceScatter, h AllGather, expert-output
    ReduceScatter) carry bf16 payloads: collective wire time dominates the
    on-device cost, and halving the bytes keeps rel err ~1.3e-3 (<< 2e-2).
"""
import os
import sys

import numpy as np

for _p in ("/opt/trn_rl_repo", "/root/.axon_site/_ro/trn_rl_repo"):
    if os.path.isdir(_p) and _p not in sys.path:
        sys.path.append(_p)

import ml_dtypes  # noqa: E402

import concourse.bacc as bacc  # noqa: E402
import concourse.bass as bass  # noqa: E402
import concourse.tile as tile  # noqa: E402
from concourse import mybir  # noqa: E402
from concourse.bass_utils import run_bass_kernel_spmd  # noqa: E402

F32 = mybir.dt.float32
F32R = mybir.dt.float32r
BF16 = mybir.dt.bfloat16
AX = mybir.AxisListType
ALU = mybir.AluOpType
ACTF = mybir.ActivationFunctionType

T = 2048          # tokens
D = 2048          # model dim
P = 128           # partitions
NT = T // P       # 16 token tiles
ND = D // P       # 16 dim chunks
HD = 128          # head dim
NQ = 16           # query heads
NE = 8            # experts
EH = 4096         # expert hidden
NEH = EH // P     # 32
NCORES = 8
RT = T // NCORES  # 256 rows per core
NRT = RT // P     # 2
EPC = 4           # experts per core (pair-wise MoE sharding)
PT = 2 * RT       # 512 tokens per core pair
EPS = 1e-6
ROPE_BASE = 5e6
NEG = -1e9
SM_SCALE = 1.0 / float(np.sqrt(HD))
HPC = NQ // NCORES   # 2 q heads per core


def _pbcast(ap, p=P):
    """AP that broadcasts a [1, ...] source across p partitions (DMA only)."""
    return bass.AP(tensor=ap.tensor, offset=ap.offset,
                   ap=[[0, p]] + [list(x) for x in ap.ap[1:]])


def _build():
    nc = bacc.Bacc()

    dp = nc.declare_dram_parameter
    x_full = dp("x_full", [T, D], F32, isOutput=False)
    x_rows = dp("x_rows", [RT, D], F32, isOutput=False)
    wqkv = dp("wqkv", [D, 512], F32R, isOutput=False)      # [Wq 2 heads | Wk | Wv]
    wo_full = dp("wo_full", [D, D], BF16, isOutput=False)   # full Wo (bf16)
    wgate = dp("wgate", [D, NE], F32, isOutput=False)
    anw = dp("anw", [1, D], F32, isOutput=False)
    fnw = dp("fnw", [1, D], F32, isOutput=False)
    qnw = dp("qnw", [1, HD], F32, isOutput=False)
    knw = dp("knw", [1, HD], F32, isOutput=False)
    cos_t = dp("cos_t", [T, HD], F32, isOutput=False)
    sin_t = dp("sin_t", [T, HD], F32, isOutput=False)
    tri01 = dp("tri01", [P, P], F32, isOutput=False)
    ident = dp("ident", [P, P], F32, isOutput=False)
    identb = dp("identb", [P, P], BF16, isOutput=False)
    esel4 = dp("esel4", [EPC, NE], F32, isOutput=False)
    onesr = dp("onesr", [P, 1], F32R, isOutput=False)
    wi_e = dp("wi_e", [EPC, NEH, P, ND, P], BF16, isOutput=False)
    wg_e = dp("wg_e", [EPC, NEH, P, ND, P], BF16, isOutput=False)
    woe = dp("woe", [EPC, ND, P, NEH, P], BF16, isOutput=False)

    out_r = dp("out_r", [RT, D], F32, isOutput=True)
    debug = bool(int(os.environ.get("DECODER_DEBUG", "0")))
    plimit = int(os.environ.get("DECODER_PHASE_LIMIT", "3"))
    if debug:
        xmid_dbg = dp("xmid_dbg", [RT, D], F32, isOutput=True)
        comb_dbg = dp("comb_dbg", [RT, NE], F32, isOutput=True)
        lgt_dbg = dp("lgt_dbg", [RT, NE], F32, isOutput=True)

    a2a_in = nc.dram_tensor("a2a_in", [T, RT], BF16)
    a2a_out = nc.dram_tensor("a2a_out", [T, RT], BF16)
    hcombT = nc.dram_tensor("hcombT", [D, RT], BF16)
    hcombT_all = nc.dram_tensor("hcombT_all", [2 * D, RT], BF16)
    combB = nc.dram_tensor("combB", [RT, NE], BF16)
    comb_all = nc.dram_tensor("comb_all", [PT, NE], BF16)
    ybuf = nc.dram_tensor("ybuf", [PT, D], BF16)
    rs2 = nc.dram_tensor("rs2", [RT, D], BF16)
    RG = [list(range(NCORES))]
    RGP = [[2 * g, 2 * g + 1] for g in range(NCORES // 2)]

    repeat = int(os.environ.get("DECODER_REPEAT", "1"))
    hwloop = int(os.environ.get("DECODER_HWLOOP", "0"))
    trace_sim = bool(int(os.environ.get("DECODER_TRACE_SIM", "0")))
    from contextlib import nullcontext

    with tile.TileContext(nc, trace_sim=trace_sim) as tc:
      with (tc.For_i(0, hwloop, 1) if hwloop else nullcontext()):
       for _rep in range(repeat):
        with (
            tc.tile_pool(name=f"consts{_rep}", bufs=1) as cp,
            tc.tile_pool(name=f"xmid{_rep}", bufs=1) as xp,
        ):
            c_ident = cp.tile([P, P], F32, tag="ident")
            nc.sync.dma_start(out=c_ident, in_=ident[:])
            c_tri = cp.tile([P, P], F32, tag="tri")
            nc.sync.dma_start(out=c_tri, in_=tri01[:])
            c_anw = cp.tile([P, D], F32, tag="anw")
            nc.gpsimd.dma_start(out=c_anw, in_=_pbcast(anw[:]))
            c_fnw = cp.tile([P, D], F32, tag="fnw")
            nc.gpsimd.dma_start(out=c_fnw, in_=_pbcast(fnw[:]))
            c_qnw = cp.tile([P, HD], F32, tag="qnw")
            nc.gpsimd.dma_start(out=c_qnw, in_=_pbcast(qnw[:]))
            c_knw = cp.tile([P, HD], F32, tag="knw")
            nc.gpsimd.dma_start(out=c_knw, in_=_pbcast(knw[:]))
            c_esel4 = cp.tile([P, EPC, NE], F32, tag="esel4")
            for _i in range(EPC):
                nc.gpsimd.dma_start(out=c_esel4[:, _i, :],
                                    in_=_pbcast(esel4[_i:_i + 1, :]))
            c_wgate = cp.tile([P, ND, NE], F32, tag="wgate")
            nc.sync.dma_start(out=c_wgate,
                              in_=wgate.rearrange("(c p) e -> p c e", p=P))
            c_ones = cp.tile([P, 1], F32R, tag="ones")
            nc.sync.dma_start(out=c_ones, in_=onesr[:])
            c_eps = cp.tile([P, 1], F32, tag="eps")
            nc.vector.memset(c_eps, EPS)
            c_ones1 = cp.tile([1, P], F32, tag="ones1")
            nc.vector.memset(c_ones1, 1.0)

            x_mid = xp.tile([P, NRT, D], F32, tag="xmid")
            from contextlib import ExitStack
            pwo_ctx = ExitStack()

            # qT/kT/vv/ctxT survive phases A..C
            if plimit == 4:
                pass
            else:
             with tc.tile_pool(name="qkv_keep", bufs=1) as pk:
                qT = pk.tile([P, HPC, T], F32R, tag="qT")    # [hd, head, tok]
                kT = pk.tile([P, T], F32R, tag="kT")         # [hd, tok]
                vv = pk.tile([P, NT, HD], F32R, tag="vv")    # [tok, kt, hd]
                ctxT = pk.tile([P, HPC, T], F32R, tag="ctxT")

                # ---------------- Phase A: rmsnorm + QKV projection ----------
                with (
                    tc.tile_pool(name="pa2", bufs=2) as pa2,
                    tc.tile_pool(name="pa1", bufs=1) as pa1,
                    tc.tile_pool(name="pas", bufs=2) as pas,
                    tc.tile_pool(name="pa_ps", bufs=2, space="PSUM") as paps,
                    tc.tile_pool(name="pa_ps2", bufs=3, space="PSUM") as paps2,
                ):
                    c_cos = pa1.tile([P, NT, HD], F32, tag="cos")
                    nc.sync.dma_start(out=c_cos,
                                      in_=cos_t.rearrange("(t p) d -> p t d", p=P))
                    c_sin = pa1.tile([P, NT, HD], F32, tag="sin")
                    nc.sync.dma_start(out=c_sin,
                                      in_=sin_t.rearrange("(t p) d -> p t d", p=P))
                    w_qkv = pa1.tile([P, ND, 512], F32R, tag="wqkv")
                    nc.sync.dma_start(out=w_qkv,
                                      in_=wqkv.rearrange("(c p) n -> p c n", p=P))
                    scr = pa1.tile([P, D], F32, tag="scr")

                    def _at_chain(tt):
                        # rmsnorm-scaled row tile; issued one tile ahead, and
                        # applied on the ACT engine (attn_norm_w is folded into
                        # the QKV weights host-side) so the wide apply doesn't
                        # clog the in-order DVE queue.
                        xt = pa2.tile([P, D], F32, tag="xt")
                        nc.sync.dma_start(out=xt,
                                          in_=x_full[tt * P:(tt + 1) * P, :])
                        ms = pas.tile([P, 1], F32, tag="ms")
                        nc.scalar.activation(out=scr, in_=xt, func=ACTF.Square,
                                             accum_out=ms)
                        nc.scalar.activation(out=ms, in_=ms, func=ACTF.Sqrt,
                                             bias=c_eps, scale=1.0 / D)
                        nc.vector.reciprocal(out=ms, in_=ms)
                        at = pa2.tile([P, D], F32, tag="at")
                        nc.scalar.activation(out=at, in_=xt, func=ACTF.Copy,
                                             scale=ms)
                        return at

                    at_cur = _at_chain(0)
                    for tt in range(NT):
                        aT = pa1.tile([P, ND, P], F32R, tag="aT")
                        for dq in range(4):
                            tp4 = paps.tile([P, 4, P], F32, tag="tp")
                            for k in range(4):
                                dc = dq * 4 + k
                                nc.tensor.transpose(
                                    out=tp4[:, k, :],
                                    in_=at_cur[:, dc * P:(dc + 1) * P],
                                    identity=c_ident)
                            nc.vector.tensor_copy(
                                out=aT[:, dq * 4:(dq + 1) * 4, :], in_=tp4)
                        qkvp = paps2.tile([P, 512], F32, tag="qkvp")
                        for dc in range(ND):
                            nc.tensor.matmul(out=qkvp[:],
                                             lhsT=aT[:, dc, :],
                                             rhs=w_qkv[:, dc, :],
                                             start=(dc == 0), stop=(dc == ND - 1))
                        if tt + 1 < NT:
                            at_next = _at_chain(tt + 1)
                        # q heads + k: per-head rmsnorm + rope, then transpose
                        for ih in range(HPC + 1):
                            seg = qkvp[:, ih * HD:(ih + 1) * HD]
                            wnorm = c_qnw if ih < HPC else c_knw
                            scr2 = pas.tile([P, HD], F32, tag="scr2")
                            ms2 = pas.tile([P, 1], F32, tag="ms2")
                            nc.scalar.activation(out=scr2, in_=seg,
                                                 func=ACTF.Square, accum_out=ms2)
                            nc.scalar.activation(out=ms2, in_=ms2,
                                                 func=ACTF.Sqrt,
                                                 bias=c_eps, scale=1.0 / HD)
                            nc.vector.reciprocal(out=ms2, in_=ms2)
                            nrm = pas.tile([P, HD], F32, tag="nrm")
                            nc.vector.scalar_tensor_tensor(
                                out=nrm, in0=seg, scalar=ms2, in1=wnorm,
                                op0=ALU.mult, op1=ALU.mult)
                            rop = pas.tile([P, HD], F32, tag="rop")
                            nc.vector.tensor_scalar_mul(
                                rop[:, :HD // 2], nrm[:, HD // 2:], -1.0)
                            nc.vector.tensor_copy(
                                out=rop[:, HD // 2:], in_=nrm[:, :HD // 2])
                            nc.vector.tensor_mul(nrm, nrm, c_cos[:, tt, :])
                            nc.vector.tensor_mul(rop, rop, c_sin[:, tt, :])
                            nc.vector.tensor_add(nrm, nrm, rop)
                            tp2 = paps.tile([P, P], F32, tag="tp")
                            nc.tensor.transpose(out=tp2, in_=nrm, identity=c_ident)
                            dst = (qT[:, ih, tt * P:(tt + 1) * P] if ih < HPC
                                   else kT[:, tt * P:(tt + 1) * P])
                            nc.vector.tensor_copy(out=dst, in_=tp2)
                        nc.vector.tensor_copy(out=vv[:, tt, :], in_=qkvp[:, 384:512])
                        if tt + 1 < NT:
                            at_cur = at_next

                # prefetch phase-C2 operands while attention runs
                # (SBUF for these frees up when the phase-A pools close)
                if plimit != 4:
                    pwo = pwo_ctx.enter_context(
                        tc.tile_pool(name=f"pwo{_rep}", bufs=1))
                    wo_sb = pwo.tile([P, ND, D], BF16, tag="wosb")
                    nc.sync.dma_start(
                        out=wo_sb,
                        in_=wo_full.rearrange("(c p) o -> p c o", p=P))
                    xr2 = pwo.tile([P, NRT, D], F32, tag="xr2")
                    nc.sync.dma_start(
                        out=xr2, in_=x_rows.rearrange("(r p) d -> p r d", p=P))

                # ---------------- Phase B: attention ----------------------
                with (
                    tc.tile_pool(name="pb", bufs=3) as pb,
                    tc.tile_pool(name="pb2", bufs=2) as pb2,
                    tc.tile_pool(name="pb_ps", bufs=2, space="PSUM") as pbps,
                    tc.tile_pool(name="pb_ps2", bufs=2, space="PSUM") as pbps2,
                    tc.tile_pool(name="pb_ps3", bufs=1, space="PSUM") as pbps3,
                ):
                    for h in range(HPC):
                        for qc in range(4):
                            cs = qc * 512
                            ctxp = pbps2.tile([P, 512], F32, tag="ctx")
                            denp = pbps3.tile([1, 512], F32, tag="den")
                            nkt = 4 * (qc + 1)
                            for kt in range(nkt):
                                lo = max(0, kt * P - cs)
                                width = 512 - lo
                                scp = pbps.tile([P, 512], F32, tag="sc")
                                nc.tensor.matmul(
                                    out=scp[:, :width],
                                    lhsT=kT[:, kt * P:(kt + 1) * P],
                                    rhs=qT[:, h, cs + lo:cs + 512],
                                    start=True, stop=True)
                                ex = pb.tile([P, 512], F32R, tag="ex")
                                nc.scalar.activation(out=ex[:, :width],
                                                     in_=scp[:, :width],
                                                     func=ACTF.Exp, scale=SM_SCALE)
                                if kt * P >= cs:
                                    # diagonal block: first 128 cols of suffix
                                    nc.vector.tensor_mul(ex[:, :P], ex[:, :P],
                                                         c_tri)
                                nc.tensor.matmul(
                                    out=ctxp[:, lo:],
                                    lhsT=vv[:, kt, :],
                                    rhs=ex[:, :width],
                                    start=(kt == 0), stop=(kt == nkt - 1))
                                nc.tensor.matmul(
                                    out=denp[:, lo:], lhsT=c_ones,
                                    rhs=ex[:, :width],
                                    start=(kt == 0), stop=(kt == nkt - 1))
                            dsb = pb2.tile([1, 512], F32, tag="dsb")
                            nc.vector.reciprocal(out=dsb, in_=denp)
                            dbc = pbps3.tile([P, 512], F32, tag="dbc")
                            nc.tensor.matmul(out=dbc[:], lhsT=c_ones1, rhs=dsb,
                                             start=True, stop=True)
                            dbc_sb = pb2.tile([P, 512], F32, tag="dbcsb")
                            nc.scalar.copy(out=dbc_sb, in_=dbc)
                            nc.vector.tensor_mul(ctxT[:, h, cs:cs + 512],
                                                 ctxp, dbc_sb)
                            # stream ctx^T out for the all-to-all as soon as
                            # this 512-token chunk of the head is final
                            for j2 in range(2):
                                j = qc * 2 + j2
                                cxb = pb.tile([P, RT], BF16, tag="cxb")
                                nc.vector.tensor_copy(
                                    out=cxb,
                                    in_=ctxT[:, h, j * RT:(j + 1) * RT])
                                nc.sync.dma_start(
                                    out=a2a_in[j * RT + h * P:
                                               j * RT + (h + 1) * P, :],
                                    in_=cxb)

                if plimit != 4:
                    nc.gpsimd.collective_compute(
                        "AllToAll", ALU.bypass, replica_groups=RG,
                        ins=[a2a_in[:]], outs=[a2a_out[:]])

                    # ------- Phase C2: x_mid = x_rows + ctx_rows @ Wo ---------
                    with (
                        tc.tile_pool(name="pc2", bufs=2) as pc2,
                        tc.tile_pool(name="pc21", bufs=1) as pc21,
                        tc.tile_pool(name="pc2_ps", bufs=2,
                                     space="PSUM") as pc2ps,
                        tc.tile_pool(name="pc2_ps2", bufs=2,
                                     space="PSUM") as pc2ps2,
                    ):
                        ctx_sb = pc21.tile([P, ND, RT], BF16, tag="ctxsb")
                        nc.sync.dma_start(
                            out=ctx_sb,
                            in_=a2a_out.rearrange("(c p) t -> p c t", p=P))
                        for do in range(ND):
                            op_ = pc2ps.tile([P, RT], F32, tag="op")
                            for dc in range(ND):
                                nc.tensor.matmul(
                                    out=op_[:],
                                    lhsT=wo_sb[:, dc, do * P:(do + 1) * P],
                                    rhs=ctx_sb[:, dc, :],
                                    start=(dc == 0), stop=(dc == ND - 1))
                            ot_sb = pc2.tile([P, RT], F32, tag="otsb")
                            nc.vector.tensor_copy(out=ot_sb, in_=op_)
                            for r in range(NRT):
                                tp = pc2ps2.tile([P, P], F32, tag="tp2")
                                nc.tensor.transpose(
                                    out=tp, in_=ot_sb[:, r * P:(r + 1) * P],
                                    identity=c_ident)
                                nc.vector.tensor_add(
                                    x_mid[:, r, do * P:(do + 1) * P],
                                    xr2[:, r, do * P:(do + 1) * P], tp)

                # wo_sb/xr2 no longer needed; free their SBUF before phase E
                pwo_ctx.close()

            if plimit >= 2 and plimit != 4:

                # ---------------- Phase D: residual, h, router ----------------
                with (
                    tc.tile_pool(name="pd", bufs=2) as pd,
                    tc.tile_pool(name="pd1", bufs=1) as pd1,
                    tc.tile_pool(name="pd_ps", bufs=2, space="PSUM") as pdps,
                    tc.tile_pool(name="pd_ps2", bufs=1, space="PSUM") as pdps2,
                ):
                    h_sb = pd1.tile([P, NRT, D], F32, tag="hsb")
                    hT_c = pd1.tile([P, ND, RT], F32, tag="hTc")
                    scr3 = pd1.tile([P, D], F32, tag="scr3")
                    for r in range(NRT):
                        ms = pd.tile([P, 1], F32, tag="ms")
                        nc.scalar.activation(out=scr3, in_=x_mid[:, r, :],
                                             func=ACTF.Square, accum_out=ms)
                        nc.scalar.activation(out=ms, in_=ms, func=ACTF.Sqrt,
                                             bias=c_eps, scale=1.0 / D)
                        nc.vector.reciprocal(out=ms, in_=ms)
                        # ffn_norm_w is folded into w_gate/wi/wg host-side, so
                        # the wide apply runs on ACT instead of clogging DVE
                        nc.scalar.activation(out=h_sb[:, r, :],
                                             in_=x_mid[:, r, :],
                                             func=ACTF.Copy, scale=ms)
                        for dq in range(4):
                            tp4 = pdps.tile([P, 4, P], F32, tag="tp")
                            for k in range(4):
                                dc = dq * 4 + k
                                nc.tensor.transpose(
                                    out=tp4[:, k, :],
                                    in_=h_sb[:, r, dc * P:(dc + 1) * P],
                                    identity=c_ident)
                            nc.vector.tensor_copy(
                                out=hT_c[:, dq * 4:(dq + 1) * 4,
                                         r * P:(r + 1) * P],
                                in_=tp4)
                        h16T = pd.tile([P, ND, P], BF16, tag="h16T")
                        nc.vector.tensor_copy(out=h16T,
                                              in_=hT_c[:, :, r * P:(r + 1) * P])
                        nc.sync.dma_start(
                            out=hcombT.rearrange(
                                "(c p) t -> p c t", p=P)[:, :,
                                                         r * P:(r + 1) * P],
                            in_=h16T)
                    # kick the big h^T AllGather before the router math; the
                    # tiny comb AllGather below only has to land ~600us later
                    # (at the yacc-scaling stage of phase E)
                    nc.gpsimd.collective_compute(
                        "AllGather", ALU.bypass, replica_groups=RGP,
                        ins=[hcombT[:]], outs=[hcombT_all[:]])
                    # router logits (plain fp32 matmuls, exact)
                    lgp = pdps2.tile([NE, RT], F32, tag="lgp")
                    for dc in range(ND):
                        nc.tensor.matmul(out=lgp[:], lhsT=c_wgate[:, dc, :],
                                         rhs=hT_c[:, dc, :],
                                         start=(dc == 0), stop=(dc == ND - 1))
                    lg_sb = pd1.tile([NE, RT], F32, tag="lgsb")
                    nc.vector.tensor_copy(out=lg_sb, in_=lgp)
                    lg_t = pd1.tile([P, NRT, NE], F32, tag="lgt")
                    for r in range(NRT):
                        tp = pdps.tile([P, NE], F32, tag="tpl")
                        nc.tensor.transpose(out=tp, in_=lg_sb[:, r * P:(r + 1) * P],
                                            identity=c_ident[:NE, :NE])
                        nc.vector.tensor_copy(out=lg_t[:, r, :], in_=tp)
                    for r in range(NRT):
                        row = lg_t[:, r, :]
                        mx = pd.tile([P, 8], F32, tag="mx")
                        nc.vector.max(out=mx, in_=row)
                        nm1 = pd.tile([P, 1], F32, tag="nm1")
                        nc.vector.tensor_scalar_mul(nm1, mx[:, 0:1], -1.0)
                        g = pd.tile([P, NE], F32, tag="g")
                        d8 = pd.tile([P, 1], F32, tag="d8")
                        nc.scalar.activation(out=g, in_=row, func=ACTF.Exp,
                                             bias=nm1, accum_out=d8)
                        nc.vector.reciprocal(out=d8, in_=d8)
                        nc.vector.tensor_scalar_mul(g, g, d8)
                        mg = pd.tile([P, 8], F32, tag="mg")
                        nc.vector.max(out=mg, in_=g)
                        msk = pd.tile([P, NE], F32, tag="msk")
                        nc.vector.tensor_scalar(out=msk, in0=g, scalar1=mg[:, 1:2],
                                                scalar2=None, op0=ALU.is_ge)
                        comb = pd.tile([P, NE], F32, tag="comb")
                        nc.vector.tensor_mul(comb, g, msk)
                        cb16 = pd.tile([P, NE], BF16, tag="cb16")
                        nc.vector.tensor_copy(out=cb16, in_=comb)
                        nc.sync.dma_start(out=combB[r * P:(r + 1) * P, :],
                                          in_=cb16)
                        if debug:
                            nc.sync.dma_start(out=comb_dbg[r * P:(r + 1) * P, :],
                                              in_=comb)
                            nc.sync.dma_start(out=lgt_dbg[r * P:(r + 1) * P, :],
                                              in_=lg_t[:, r, :])
                            nc.sync.dma_start(out=xmid_dbg[r * P:(r + 1) * P, :],
                                              in_=x_mid[:, r, :])

                nc.gpsimd.collective_compute(
                    "AllGather", ALU.bypass, replica_groups=RGP,
                    ins=[combB[:]], outs=[comb_all[:]])

            if plimit in (1, 2):
                with tc.tile_pool(name="px1", bufs=2) as px1:
                    for r in range(NRT):
                        nc.sync.dma_start(out=out_r[r * P:(r + 1) * P, :],
                                          in_=x_mid[:, r, :])
            if plimit >= 3:
                # ------- Phase E: 4 experts x 512 pair-local tokens ----------
                with (
                    tc.tile_pool(name="pe1", bufs=1) as pe1,
                    tc.tile_pool(name="pew", bufs=3) as pew,
                    tc.tile_pool(name="pes", bufs=2) as pes,
                    tc.tile_pool(name="pes1", bufs=1) as pes1,
                    tc.tile_pool(name="pe_ps", bufs=2, space="PSUM") as peps,
                    tc.tile_pool(name="pe_ps2", bufs=2, space="PSUM") as peps2,
                    tc.tile_pool(name="pe_ps3", bufs=2, space="PSUM") as peps3,
                ):
                    NJ = PT // P    # 4 token tiles of 128
                    hT_g = pe1.tile([P, ND, PT], BF16, tag="hTg")
                    act_g = pe1.tile([P, NEH, PT], BF16, tag="actg")
                    combg = pe1.tile([P, NJ, EPC], F32, tag="combg")
                    yacc = pe1.tile([P, NJ, D], F32, tag="yacc")
                    for half in range(2):
                        # shard `half` of the pair AG = that core's 256 rows,
                        # already transposed: load straight into the matmul rhs
                        nc.sync.dma_start(
                            out=hT_g[:, :, half * RT:(half + 1) * RT],
                            in_=hcombT_all.rearrange(
                                "(s c p) t -> s p c t", s=2, p=P)[half])
                    for j in range(NJ):
                        tt0 = j * P
                        cbl = pes.tile([P, NE], BF16, tag="cbl")
                        nc.sync.dma_start(out=cbl,
                                          in_=comb_all[tt0:tt0 + P, :])
                        for ei in range(EPC):
                            cbm = pes.tile([P, NE], F32, tag="cbm")
                            nc.vector.tensor_mul(cbm, cbl, c_esel4[:, ei, :])
                            nc.vector.tensor_reduce(
                                out=combg[:, j, ei:ei + 1], in_=cbm,
                                axis=AX.X, op=ALU.add)
                    for ei in range(EPC):
                        for et in range(NEH):
                            wi_s = pew.tile([P, ND, P], BF16, tag="wis")
                            nc.sync.dma_start(out=wi_s, in_=wi_e[ei, et])
                            wg_s = pew.tile([P, ND, P], BF16, tag="wgs")
                            nc.sync.dma_start(out=wg_s, in_=wg_e[ei, et])
                            upp = peps3.tile([P, 512], F32, tag="upp")
                            gtp = peps2.tile([P, 512], F32, tag="peb")
                            for dc in range(ND):
                                nc.tensor.matmul(
                                    out=upp[:], lhsT=wi_s[:, dc, :],
                                    rhs=hT_g[:, dc, :],
                                    start=(dc == 0), stop=(dc == ND - 1))
                            for dc in range(ND):
                                nc.tensor.matmul(
                                    out=gtp[:], lhsT=wg_s[:, dc, :],
                                    rhs=hT_g[:, dc, :],
                                    start=(dc == 0), stop=(dc == ND - 1))
                            sil = pes.tile([P, 512], BF16, tag="sil")
                            nc.scalar.activation(out=sil, in_=gtp, func=ACTF.Silu)
                            nc.vector.tensor_tensor(
                                out=act_g[:, et, :], in0=sil, in1=upp,
                                op=ALU.mult)
                        for dt in range(ND):
                            wo_s = pew.tile([P, NEH, P], BF16, tag="wos")
                            nc.sync.dma_start(out=wo_s, in_=woe[ei, dt])
                            yp = peps.tile([P, 512], F32, tag="pea")
                            for ec in range(NEH):
                                nc.tensor.matmul(
                                    out=yp[:], lhsT=wo_s[:, ec, :],
                                    rhs=act_g[:, ec, :],
                                    start=(ec == 0), stop=(ec == NEH - 1))
                            ysb = pes.tile([P, 512], F32, tag="ysb")
                            nc.vector.tensor_copy(out=ysb, in_=yp)
                            for q in range(NJ):
                                tp = peps2.tile([P, 512], F32, tag="peb")
                                nc.tensor.transpose(
                                    out=tp[:, :P], in_=ysb[:, q * P:(q + 1) * P],
                                    identity=c_ident)
                                dst = yacc[:, q, dt * P:(dt + 1) * P]
                                if ei == 0:
                                    nc.vector.tensor_scalar_mul(
                                        dst, tp[:, :P], combg[:, q, 0:1])
                                else:
                                    nc.vector.scalar_tensor_tensor(
                                        out=dst, in0=tp[:, :P],
                                        scalar=combg[:, q, ei:ei + 1],
                                        in1=dst, op0=ALU.mult, op1=ALU.add)
                    for j in range(NJ):
                        y16 = pes.tile([P, D], BF16, tag="y16")
                        nc.vector.tensor_copy(out=y16, in_=yacc[:, j, :])
                        nc.sync.dma_start(out=ybuf[j * P:(j + 1) * P, :],
                                          in_=y16)

                if plimit != 4:
                    nc.gpsimd.collective_compute(
                        "ReduceScatter", ALU.add, replica_groups=RGP,
                        ins=[ybuf[:]], outs=[rs2[:]])

                # ---------------- Phase F: final residual ---------------------
                with tc.tile_pool(name="pf", bufs=2) as pf:
                    for r in range(NRT):
                        rr = pf.tile([P, D], BF16, tag="rr2")
                        src_t = ybuf if plimit == 4 else rs2
                        nc.sync.dma_start(out=rr, in_=src_t[r * P:(r + 1) * P, :])
                        ot = pf.tile([P, D], F32, tag="ot")
                        if plimit == 4:
                            nc.vector.tensor_copy(out=ot, in_=rr)
                            nc.sync.dma_start(out=out_r[r * P:(r + 1) * P, :],
                                              in_=ot)
                        else:
                            nc.vector.tensor_add(ot, x_mid[:, r, :], rr)
                            nc.sync.dma_start(out=out_r[r * P:(r + 1) * P, :],
                                              in_=ot)


    nc.finalize()
    return nc, debug


_PROG = None


def _get_prog():
    global _PROG
    if _PROG is None:
        _PROG = _build()
    return _PROG


def _rope_tables():
    inv_freq = 1.0 / (ROPE_BASE ** (np.arange(0, HD, 2, dtype=np.float32) / HD))
    t = np.arange(T, dtype=np.float32)
    freqs = np.einsum("i,j->ij", t, inv_freq).astype(np.float32)
    emb = np.concatenate((freqs, freqs), axis=-1)
    return np.cos(emb).astype(np.float32), np.sin(emb).astype(np.float32)


def _wtile_in(w):
    """[D, EH] -> [NEH, P, ND, P] bf16: contiguous per-et lhsT strips."""
    return np.ascontiguousarray(
        w.reshape(ND, P, NEH, P).transpose(2, 1, 0, 3)
    ).astype(ml_dtypes.bfloat16)


def _wtile_out(w):
    """[EH, D] -> [ND, P, NEH, P] bf16: contiguous per-dt lhsT strips."""
    return np.ascontiguousarray(
        w.reshape(NEH, P, ND, P).transpose(2, 1, 0, 3)
    ).astype(ml_dtypes.bfloat16)


_PREP_CACHE = {}


def _make_in_maps(inputs):
    x = np.ascontiguousarray(np.asarray(inputs["x"], np.float32).reshape(T, D))
    mask = np.asarray(inputs["attn_mask"], np.float32).reshape(T, T)
    causal = np.triu(np.full((T, T), NEG, np.float32), k=1)
    if not np.array_equal(mask, causal):
        raise NotImplementedError("kernel compiled for the causal attn_mask")

    Wq = np.asarray(inputs["Wq"], np.float32)
    Wk = np.asarray(inputs["Wk"], np.float32)
    Wv = np.asarray(inputs["Wv"], np.float32)
    Wo = np.asarray(inputs["Wo"], np.float32)
    wi = np.asarray(inputs["wi"], np.float32)
    wg = np.asarray(inputs["wg"], np.float32)
    wo = np.asarray(inputs["wo"], np.float32)
    cos_np, sin_np = _rope_tables()
    tri = np.triu(np.ones((P, P), np.float32))           # [k, q]: 1 if q >= k
    ident_np = np.eye(P, dtype=np.float32)

    key = (np.asarray(inputs["wi"]).ctypes.data,
           np.asarray(inputs["x"]).ctypes.data)
    cached = _PREP_CACHE.get(key)
    if cached is not None:
        return cached
    fnw_col = np.asarray(inputs["ffn_norm_w"], np.float32).reshape(D, 1)
    wi_all = np.stack([_wtile_in(wi[e] * fnw_col) for e in range(NE)])
    wg_all = np.stack([_wtile_in(wg[e] * fnw_col) for e in range(NE)])
    wo_all = np.stack([_wtile_out(wo[e]) for e in range(NE)])
    Wo_b16 = np.ascontiguousarray(Wo).astype(ml_dtypes.bfloat16)
    in_maps = []
    for c in range(NCORES):
        g = c // 2
        anw_col = np.asarray(inputs["attn_norm_w"],
                             np.float32).reshape(D, 1)
        wqkv_c = np.ascontiguousarray(np.concatenate(
            [Wq[:, 2 * c * HD:(2 * c + 2) * HD],
             Wk[:, g * HD:(g + 1) * HD],
             Wv[:, g * HD:(g + 1) * HD]], axis=1) * anw_col)
        e0 = EPC * (c % 2)
        esel4_c = np.zeros((EPC, NE), np.float32)
        for i in range(EPC):
            esel4_c[i, e0 + i] = 1.0
        in_maps.append({
            "x_full": x,
            "x_rows": np.ascontiguousarray(x[c * RT:(c + 1) * RT, :]),
            "wqkv": wqkv_c,
            "wo_full": Wo_b16,
            "wgate": np.ascontiguousarray(
                np.asarray(inputs["w_gate"], np.float32) * fnw_col),
            "anw": np.asarray(inputs["attn_norm_w"], np.float32).reshape(1, D),
            "fnw": np.asarray(inputs["ffn_norm_w"], np.float32).reshape(1, D),
            "qnw": np.asarray(inputs["q_norm_w"], np.float32).reshape(1, HD),
            "knw": np.asarray(inputs["k_norm_w"], np.float32).reshape(1, HD),
            "cos_t": cos_np,
            "sin_t": sin_np,
            "tri01": tri,
            "ident": ident_np,
            "identb": ident_np.astype(ml_dtypes.bfloat16),
            "esel4": esel4_c,
            "onesr": np.ones((P, 1), np.float32),
            "wi_e": wi_all[e0:e0 + EPC],
            "wg_e": wg_all[e0:e0 + EPC],
            "woe": wo_all[e0:e0 + EPC],
        })
    return in_maps


_RUNNER = None


def _get_runner():
    """Persistent jitted SPMD executor (compiles once per process)."""
    global _RUNNER
    if _RUNNER is None:
        import jax
        from jax.experimental.shard_map import shard_map
        from jax.sharding import Mesh, PartitionSpec

        from concourse import bass2jax as b2j

        nc, debug = _get_prog()
        b2j.install_neuronx_cc_hook()
        pname = nc.partition_id_tensor.name if nc.partition_id_tensor else None
        in_names, out_names, out_avals, zero_specs = [], [], [], []
        for alloc in nc.m.functions[0].allocations:
            if not isinstance(alloc, mybir.MemoryLocationSet):
                continue
            name = alloc.memorylocations[0].name
            if alloc.kind == "ExternalInput":
                if name != pname:
                    in_names.append(name)
            elif alloc.kind == "ExternalOutput":
                out_names.append(name)
                shape = tuple(alloc.tensor_shape)
                dt_np = mybir.dt.np(alloc.dtype)
                out_avals.append(jax.core.ShapedArray(shape, dt_np))
                zero_specs.append((shape, dt_np))
        n_params = len(in_names)
        all_in = list(in_names) + list(out_names) + ([pname] if pname else [])
        donate = tuple(range(n_params, n_params + len(out_names)))

        def _body(*args):
            operands = list(args)
            if pname is not None:
                operands.append(b2j.partition_id_tensor())
            outs = b2j._bass_exec_p.bind(
                *operands, out_avals=tuple(out_avals), in_names=tuple(all_in),
                out_names=tuple(out_names), lowering_input_output_aliases=(),
                sim_require_finite=True, sim_require_nnan=True, nc=nc)
            return tuple(outs)

        devices = jax.devices()[:NCORES]
        mesh = Mesh(np.asarray(devices), ("core",))
        nio = n_params + len(out_names)
        sharded = jax.jit(
            shard_map(_body, mesh=mesh, in_specs=(PartitionSpec("core"),) * nio,
                      out_specs=(PartitionSpec("core"),) * len(out_names),
                      check_rep=False),
            donate_argnums=donate, keep_unused=True)
        _RUNNER = (sharded, in_names, out_names, zero_specs, debug)
    return _RUNNER


def _run(in_maps):
    sharded, in_names, out_names, zero_specs, debug = _get_runner()
    concat_in = [
        np.concatenate([np.asarray(in_maps[c][nm]) for c in range(NCORES)],
                       axis=0)
        for nm in in_names
    ]
    zeros = [np.zeros((NCORES * s[0],) + tuple(s[1:]), d)
             for (s, d) in zero_specs]
    outs = sharded(*concat_in, *zeros)
    return {nm: np.asarray(outs[i]) for i, nm in enumerate(out_names)}, debug


def kernel(**inputs):
    in_maps = _make_in_maps(inputs)
    res, debug = _run(in_maps)
    out = res["out_r"]  # [NCORES*RT, D] = [T, D], rank-concat = token order
    if debug:
        kernel._dbg = res
    return out.reshape(1, T, D).astype(np.float32)

